# revision 8
# baseline (speedup 1.0000x reference)
"""Trainium2 Bass kernel for nn_CutLayer (histogram_binning).

Two device launches over 8 cores (data-parallel on events):

L1 "counts": per-core class-compacted bf16 tile [128, F2] (rows 0-63 =
  signal events, rows 64-127 = background, pads = +BIG). 51 bf16-grid
  edge thresholds are counted by three engine paths running concurrently:
    - PE path (N_B edges): DVE plain tensor_scalar compare at 4x rate
      (bf16) into ping-pong buffers; PE reduces each compare tile with a
      one-hot stationary matmul into PSUM rows [2 per edge], accumulated
      across 512-column chunks; one final DVE reduce -> [2*N_B, 1].
    - DVE-accum path (N_A edges): fused tensor_scalar compare+accum (1x).
    - ACT path (N_C edges): Sign activation with bias strictly between
      bf16 grid points (no sign(0) ties) + accumulator.
  Host converts to exact fp32-semantics counts (le and lt) by correcting
  a small candidate set of events within a few bf16 ulps of each edge,
  then replicates the reference's pair search bit-exactly (eager CPU jax)
  to produce (lower, upper, case).

L2 "pred": case-specialized bf16 compare(s) on the original-order
  events; host flips the few events within bf16 rounding of the chosen
  cuts and handles the 512-event layout tail exactly.

Host work is O(N) numpy prep/fixup only: min/max, bf16 casts, class
compaction, candidate repair, tiny 51x51 pair search.
"""

from contextlib import ExitStack

import numpy as np
import ml_dtypes

import concourse.bass as bass
import concourse.mybir as mybir
from concourse.bass_utils import run_bass_kernel_spmd

N = 8_000_000
N_CORES = 8
CORE_N = N // N_CORES            # 1_000_000
P = 128
HP = 64                          # rows per class in the counts tile
N_BINS = 50
E = N_BINS + 1                   # 51 edges
EPS = 1e-7
BIG = np.float32(1.0e30)         # bf16-exact sentinel above every edge

# pred layout (original order)
FP = 7812
DEV_N = P * FP                   # 999_936 device events/core for pred

# counts engine split (sums to E)
N_B = 27                         # DVE compare -> PE matmul reduce
N_A = 7                          # DVE fused compare+accum
N_C = 17                         # ACT sign path
N_F = 15                         # of the N_B edges, pool-folded ones
CH = 512                         # psum chunk columns

FP32 = mybir.dt.float32
BF16 = mybir.dt.bfloat16
I32 = mybir.dt.int32
AX = mybir.AxisListType
OP = mybir.AluOpType
ACT = mybir.ActivationFunctionType

CORE_IDS = list(range(N_CORES))
BF = ml_dtypes.bfloat16


# --------------------------------------------------------------------------
# Bass programs
# --------------------------------------------------------------------------

def _build_counts_v2(F2: int):
    nchunks = (F2 + CH - 1) // CH
    H2 = F2 // 2
    nhalf = (H2 + CH - 1) // CH
    MROWS = 2 * N_B
    # which B-edges are pool-folded (PE does half the matmuls on those)
    fold = [(i * N_F) // N_B > ((i - 1) * N_F) // N_B for i in range(N_B)]
    fidx = {}
    uidx = {}
    for i in range(N_B):
        if fold[i]:
            fidx[i] = len(fidx)
        else:
            uidx[i] = len(uidx)
    nc = bass.Bass()
    xt_in = nc.declare_dram_parameter("xt", [P, F2], BF16, isOutput=False)
    ed = nc.declare_dram_parameter("edges", [P, 2 * E], FP32, isOutput=False)
    oh_in = nc.declare_dram_parameter("oh", [P, N_B * MROWS], BF16,
                                      isOutput=False)
    acca_o = nc.declare_dram_parameter("acca", [P, N_A], FP32, isOutput=True)
    accc_o = nc.declare_dram_parameter("accc", [P, N_C], FP32, isOutput=True)
    accb_o = nc.declare_dram_parameter("accb", [MROWS, 1], FP32, isOutput=True)
    with ExitStack() as es:
        ec = es.enter_context
        xt = ec(nc.sbuf_tensor([P, F2], BF16))
        cb = [ec(nc.sbuf_tensor(f"cb{i}", [P, F2], BF16)) for i in range(2)]
        fb = [ec(nc.sbuf_tensor(f"fb{i}", [P, H2], BF16)) for i in range(2)]
        scra = ec(nc.sbuf_tensor([P, F2], BF16))
        scrc = ec(nc.sbuf_tensor([P, F2], BF16))
        edt = ec(nc.sbuf_tensor([P, 2 * E], FP32))
        oht = ec(nc.sbuf_tensor([P, N_B * MROWS], BF16))
        acca = ec(nc.sbuf_tensor([P, N_A], FP32))
        accc = ec(nc.sbuf_tensor([P, N_C], FP32))
        accb = ec(nc.sbuf_tensor([MROWS, 1], FP32))
        ps = ec(nc.psum_tensor([MROWS, CH], FP32))
        dse = ec(nc.semaphore("dse"))
        dsx0 = ec(nc.semaphore("dsx0"))
        dsx1 = ec(nc.semaphore("dsx1"))
        vprod = ec(nc.semaphore("vprod"))
        ucons = ec(nc.semaphore("ucons"))   # PE consumed unfolded edge
        pfold = ec(nc.semaphore("pfold"))   # pool folded edge f (also frees cb)
        fdone = ec(nc.semaphore("fdone"))   # PE consumed fold f
        adone = ec(nc.semaphore("adone"))
        cdone = ec(nc.semaphore("cdone"))
        bdone = ec(nc.semaphore("bdone"))
        dso = ec(nc.semaphore("dso"))
        block = ec(nc.Block())

        @block.sync
        def _(sync):
            sync.wait_ge(cdone, 1)
            sync.dma_start(accc_o[:], accc[:]).then_inc(dso, 16)
            sync.wait_ge(adone, 1)
            sync.dma_start(acca_o[:], acca[:]).then_inc(dso, 16)
            sync.wait_ge(bdone, 1)
            sync.dma_start(accb_o[:], accb[:]).then_inc(dso, 16)
            sync.wait_ge(dso, 48)

        @block.scalar
        def _(scalar):
            scalar.dma_start(edt[:], ed[:]).then_inc(dse, 16)
            scalar.dma_start(oht[:], oh_in[:]).then_inc(dse, 16)
            scalar.dma_start(xt[:, 0:H2], xt_in[:, 0:H2]).then_inc(dsx0, 16)
            scalar.wait_ge(dse, 32)
            scalar.wait_ge(dsx0, 16)
            scalar.wait_ge(dsx1, 16)
            for j in range(N_C):
                ins = scalar.activation(
                    scrc[:], xt[:], ACT.Sign, bias=edt[:, E + j : E + j + 1],
                    scale=1.0, accum_out=accc[:, j : j + 1],
                )
                if j == N_C - 1:
                    ins.then_inc(cdone, 1)

        @block.vector
        def _(vector):
            vector.wait_ge(dse, 32)
            vector.wait_ge(dsx0, 16)
            vector.wait_ge(dsx1, 16)
            na = 0
            last_a_ins = None
            for i in range(N_B):
                j = i - 2
                if j >= 0:
                    if fold[j]:
                        vector.wait_ge(pfold, fidx[j] + 1)
                    else:
                        vector.wait_ge(ucons, uidx[j] + 1)
                vector.tensor_scalar(
                    cb[i % 2][:], xt[:], edt[:, N_A + i : N_A + i + 1], None,
                    OP.is_le,
                ).then_inc(vprod, 1)
                if (i % 4 == 3) and na < N_A:
                    last_a_ins = vector.tensor_scalar(
                        scra[:], xt[:], edt[:, na : na + 1], 0.0,
                        OP.is_le, OP.add, accum_out=acca[:, na : na + 1],
                    )
                    na += 1
            while na < N_A:
                last_a_ins = vector.tensor_scalar(
                    scra[:], xt[:], edt[:, na : na + 1], 0.0,
                    OP.is_le, OP.add, accum_out=acca[:, na : na + 1],
                )
                na += 1
            if last_a_ins is not None:
                last_a_ins.then_inc(adone, 1)
            vector.wait_ge(ucons, len(uidx))
            vector.wait_ge(fdone, len(fidx))
            vector.tensor_reduce(
                accb[:, 0:1], ps[:, 0:CH], axis=AX.X, op=OP.add
            ).then_inc(bdone, 1)

        @block.gpsimd
        def _(gpsimd):
            gpsimd.dma_start(xt[:, H2:F2], xt_in[:, H2:F2]).then_inc(dsx1, 16)
            for i in range(N_B):
                if not fold[i]:
                    continue
                f = fidx[i]
                gpsimd.wait_ge(vprod, i + 1)
                if f >= 2:
                    gpsimd.wait_ge(fdone, f - 1)
                gpsimd.tensor_tensor(
                    fb[f % 2][:], cb[i % 2][:, 0:H2], cb[i % 2][:, H2:F2],
                    OP.add,
                ).then_inc(pfold, 1)

        @block.tensor
        def _(tensor):
            first = True
            for i in range(N_B):
                w = oht[:, i * MROWS : (i + 1) * MROWS]
                if fold[i]:
                    f = fidx[i]
                    tensor.wait_ge(pfold, f + 1)
                    src_t = fb[f % 2]
                    nck = nhalf
                    cap = H2
                else:
                    tensor.wait_ge(vprod, i + 1)
                    src_t = cb[i % 2]
                    nck = nchunks
                    cap = F2
                for c in range(nck):
                    c0 = c * CH
                    c1 = min(cap, c0 + CH)
                    ins = tensor.matmul(
                        ps[:, 0 : c1 - c0], w, src_t[:, c0:c1],
                        start=first,
                        stop=(i == N_B - 1 and c == nck - 1),
                        skip_group_check=True,
                    )
                    if c > 0:
                        ins.ins.ldweights = False
                    first = False
                if fold[i]:
                    ins.then_inc(fdone, 1)
                else:
                    ins.then_inc(ucons, 1)
    return nc


def _build_pred(case: int):
    """Case-specialized predicate on bf16 events (original order):
    0: x <= lo ; 1: x >= lo ; 2: (x >= lo) & (x <= up) ;
    3: (x <= lo) | (x >= up)  (disjoint -> add)
    """
    nc = bass.Bass()
    x = nc.declare_dram_parameter("x", [DEV_N], BF16, isOutput=False)
    pr = nc.declare_dram_parameter("prm", [P, 8], FP32, isOutput=False)
    out = nc.declare_dram_parameter("pred", [DEV_N], BF16, isOutput=True)
    HF = FP // 2
    with (
        nc.sbuf_tensor([P, FP], BF16) as xt,
        nc.sbuf_tensor([P, FP], BF16) as t,
        nc.sbuf_tensor([P, FP], BF16) as s,
        nc.sbuf_tensor([P, FP], BF16) as pi,
        nc.sbuf_tensor([P, 8], FP32) as prm,
        nc.semaphore("d0") as d0,
        nc.semaphore("d1") as d1,
        nc.semaphore("csem") as csem,
        nc.semaphore("dso") as dso,
        nc.Block() as block,
    ):
        xv = x[:].rearrange("(p f) -> p f", p=P)
        ov = out[:].rearrange("(p f) -> p f", p=P)

        @block.sync
        def _(sync):
            sync.wait_ge(dso, 32)

        @block.scalar
        def _(scalar):
            scalar.dma_start(prm[:], pr[:]).then_inc(d0, 16)
            scalar.dma_start(xt[:, 0:HF], xv[:, 0:HF]).then_inc(d0, 16)
            scalar.wait_ge(csem, 1)
            scalar.dma_start(ov[:, 0:HF], pi[:, 0:HF]).then_inc(dso, 16)
            scalar.wait_ge(csem, 2)
            scalar.dma_start(ov[:, HF:FP], pi[:, HF:FP]).then_inc(dso, 16)

        @block.gpsimd
        def _(gpsimd):
            gpsimd.dma_start(xt[:, HF:FP], xv[:, HF:FP]).then_inc(d1, 16)

        @block.vector
        def _(vector):
            lo = prm[:, 0:1]
            up = prm[:, 1:2]
            vector.wait_ge(d0, 32)
            for h in (0, 1):
                if h == 1:
                    vector.wait_ge(d1, 16)
                sl = slice(0, HF) if h == 0 else slice(HF, FP)
                if case == 0:
                    vector.tensor_scalar(
                        pi[:, sl], xt[:, sl], lo, None, OP.is_le
                    ).then_inc(csem, 1)
                elif case == 1:
                    vector.tensor_scalar(
                        pi[:, sl], xt[:, sl], lo, None, OP.is_ge
                    ).then_inc(csem, 1)
                elif case == 2:
                    vector.tensor_scalar(t[:, sl], xt[:, sl], up, None,
                                         OP.is_le)
                    vector.tensor_scalar(s[:, sl], xt[:, sl], lo, None,
                                         OP.is_ge)
                    vector.tensor_tensor(
                        pi[:, sl], s[:, sl], t[:, sl], OP.mult
                    ).then_inc(csem, 1)
                else:
                    vector.tensor_scalar(t[:, sl], xt[:, sl], up, None,
                                         OP.is_ge)
                    vector.tensor_scalar(s[:, sl], xt[:, sl], lo, None,
                                         OP.is_le)
                    vector.tensor_tensor(
                        pi[:, sl], s[:, sl], t[:, sl], OP.add
                    ).then_inc(csem, 1)
    return nc


_PROGRAMS: dict = {}


def _prog(name, *args):
    key = (name, args)
    if key not in _PROGRAMS:
        if name == "counts":
            _PROGRAMS[key] = _build_counts_v2(*args)
        else:
            _PROGRAMS[key] = _build_pred(int(name[4:]))
    return _PROGRAMS[key]


LAST_EXEC_NS: list = []
_CACHE_SET = False


def _enable_jit_cache():
    global _CACHE_SET
    if _CACHE_SET:
        return
    _CACHE_SET = True
    try:
        import jax

        jax.config.update("jax_compilation_cache_dir", "/tmp/jax_bass_cache")
        jax.config.update("jax_persistent_cache_min_compile_time_secs", 1.0)
        jax.config.update("jax_persistent_cache_min_entry_size_bytes", 0)
    except Exception:
        pass


def _run(name, in_maps, *args):
    import os

    _enable_jit_cache()
    trace = bool(int(os.environ.get("BASS_KERNEL_PROFILE", "0")))
    r = run_bass_kernel_spmd(_prog(name, *args), in_maps, CORE_IDS, trace=trace)
    if trace:
        LAST_EXEC_NS.append((name, r.exec_time_ns, r.mean_exec_time_ns))
    return r.results


# --------------------------------------------------------------------------
# Host orchestration
# --------------------------------------------------------------------------

def _ulp_quarter(e64):
    """0.25 * (lower bound of the bf16 ulp at e), elementwise, float64."""
    a = np.abs(e64)
    a = np.where(a < 1e-30, 1e-30, a)
    return 0.25 * np.exp2(np.floor(np.log2(a)) - 8.0)


def kernel(inputs: np.ndarray, targets: np.ndarray) -> np.ndarray:
    import jax
    import jax.numpy as jnp

    x = np.ascontiguousarray(inputs[:, 0]).astype(np.float32, copy=False)
    y = np.asarray(targets)
    sig = y == 1

    LAST_EXEC_NS.clear()

    # ---- edges (host, bit-exact with the reference) -----------------------
    gmin = np.float32(x.min())
    gmax = np.float32(x.max())
    cpu = jax.devices("cpu")[0]
    with jax.default_device(cpu):
        edges = np.asarray(jnp.linspace(jnp.float32(gmin), jnp.float32(gmax), E))
    e_cmp = edges.astype(BF).astype(np.float32)          # bf16-grid thresholds
    e_act64 = e_cmp.astype(np.float64) + _ulp_quarter(e_cmp.astype(np.float64))
    e_act = e_act64.astype(np.float32)                   # strictly between grid pts

    # ---- class-compacted bf16 tiles --------------------------------------
    xb_all = x.astype(BF)
    xs_sig = xb_all[sig]
    xs_bg = xb_all[~sig]
    Ns_i, Nb_i = xs_sig.size, xs_bg.size
    cap_rows = N_CORES * HP
    F2 = (max(Ns_i, Nb_i) + cap_rows - 1) // cap_rows
    if F2 % 2:
        F2 += 1
    big_bf = BF(BIG)
    sig_pad = np.full(cap_rows * F2, big_bf, dtype=BF)
    sig_pad[:Ns_i] = xs_sig
    bg_pad = np.full(cap_rows * F2, big_bf, dtype=BF)
    bg_pad[:Nb_i] = xs_bg
    sig_tiles = sig_pad.reshape(N_CORES, HP, F2)
    bg_tiles = bg_pad.reshape(N_CORES, HP, F2)

    # edge -> slot assignment: A slots 0..N_A-1, B slots N_A..N_A+N_B-1 use
    # e_cmp; C slots use -e_act. Keep natural order (all e_act distinct from
    # e_cmp by construction; _ulp_quarter guards tiny magnitudes).
    ed_in = np.zeros((P, 2 * E), np.float32)
    ed_in[:, :E] = np.concatenate([e_cmp[:N_A], e_cmp[N_A : N_A + N_B],
                                   np.zeros(E - N_A - N_B, np.float32)])
    ed_in[:, E : E + N_C] = -e_act[N_A + N_B :]

    MROWS = 2 * N_B
    oh = np.zeros((P, N_B * MROWS), BF)
    for i in range(N_B):
        oh[0:HP, i * MROWS + 2 * i] = 1        # signal rows -> psum row 2i
        oh[HP:P, i * MROWS + 2 * i + 1] = 1    # background rows -> 2i+1

    res = _run(
        "counts",
        [
            {
                "xt": np.ascontiguousarray(
                    np.concatenate([sig_tiles[c], bg_tiles[c]], axis=0)
                ),
                "edges": ed_in,
                "oh": oh,
            }
            for c in CORE_IDS
        ],
        F2,
    )

    # ---- decode device counts (counts of xb <= e_cmp[k], per class) ------
    d_sig = np.zeros(E, np.float64)
    d_bg = np.zeros(E, np.float64)
    TOT_HALF = N_CORES * HP * F2
    for r in res:
        a = r["acca"].astype(np.float64)      # [P, N_A]
        c = r["accc"].astype(np.float64)      # [P, N_C]
        b = r["accb"].astype(np.float64)      # [MROWS, 1]
        d_sig[0:N_A] += a[0:HP].sum(axis=0)
        d_bg[0:N_A] += a[HP:P].sum(axis=0)
        d_sig[N_A : N_A + N_B] += b[0::2, 0]
        d_bg[N_A : N_A + N_B] += b[1::2, 0]
        # ACT: S = sum sign(x - e_act); le = (TOT - S)/2 per class half
        d_sig[N_A + N_B :] -= c[0:HP].sum(axis=0) / 2.0
        d_bg[N_A + N_B :] -= c[HP:P].sum(axis=0) / 2.0
    d_sig[N_A + N_B :] += TOT_HALF / 2.0
    d_bg[N_A + N_B :] += TOT_HALF / 2.0
    # pads count as "greater" in every path: subtract nothing for le counts.

    # ---- host repair: exact le/lt counts under fp32 semantics ------------
    h64 = (np.float64(gmax) - np.float64(gmin)) / N_BINS
    inv_h = np.float32(1.0 / h64) if h64 != 0 else np.float32(0.0)
    u = (x - gmin) * inv_h
    k0 = np.rint(u).astype(np.int64)
    tol = (6.0 / 128.0) * (np.abs(edges.astype(np.float64)) + 0.01)
    cand_mask = np.zeros(N, bool)
    for dk in (-1, 0, 1):
        kk = np.clip(k0 + dk, 0, E - 1)
        cand_mask |= np.abs(x.astype(np.float64) - edges[kk]) <= tol[kk]
    ci = np.flatnonzero(cand_mask)
    cx = x[ci]
    cxb = xb_all[ci].astype(np.float32)
    csig = sig[ci]
    ck0 = k0[ci]

    corr_le_sig = np.zeros(E, np.float64)
    corr_le_bg = np.zeros(E, np.float64)
    corr_lt_sig = np.zeros(E, np.float64)
    corr_lt_bg = np.zeros(E, np.float64)
    for dk in (-1, 0, 1):
        kr = ck0 + dk
        use = (kr >= 0) & (kr < E)
        kku = kr[use]
        dev = (cxb[use] <= e_cmp[kku]).astype(np.float64)
        t_le = (cx[use] <= edges[kku]).astype(np.float64)
        t_lt = (cx[use] < edges[kku]).astype(np.float64)
        s_u = csig[use]
        np.add.at(corr_le_sig, kku[s_u], (t_le - dev)[s_u])
        np.add.at(corr_le_bg, kku[~s_u], (t_le - dev)[~s_u])
        np.add.at(corr_lt_sig, kku[s_u], (t_lt - dev)[s_u])
        np.add.at(corr_lt_bg, kku[~s_u], (t_lt - dev)[~s_u])

    ns_le = (d_sig + corr_le_sig).astype(np.float32)
    nb_le = (d_bg + corr_le_bg).astype(np.float32)
    ns_lt = (d_sig + corr_lt_sig).astype(np.float32)
    nb_lt = (d_bg + corr_lt_bg).astype(np.float32)

    # ---- replicate the reference's pair search bit-exactly ----------------
    with jax.default_device(cpu):
        ns_le_j = jnp.asarray(ns_le)
        ns_lt_j = jnp.asarray(ns_lt)
        nb_le_j = jnp.asarray(nb_le)
        nb_lt_j = jnp.asarray(nb_lt)
        n_f = jnp.float32(N)
        Ns = ns_le_j[-1]
        Nb = n_f - Ns

        hist0 = nb_le_j[1:] - nb_lt_j[:-1]
        hist1 = ns_le_j[1:] - ns_lt_j[:-1]

        gt0 = hist0 > hist1
        cand0 = jnp.logical_xor(gt0[:-1], gt0[1:]) & (hist0[:-1] > 0)
        gt1 = hist1 > hist0
        cand1 = jnp.logical_xor(gt1[:-1], gt1[1:]) & (hist1[:-1] > 0)
        mask = jnp.zeros((E,), bool).at[1:N_BINS].set(cand0 | cand1)
        cnt = jnp.sum(mask)
        mask = mask.at[-1].set(mask[-1] | (cnt == 1))

        a_c = -jnp.log1p(jnp.float32(-EPS))
        b_c = -jnp.log(jnp.float32(EPS))

        def bce(correct):
            return ((n_f - correct) * b_c + correct * a_c) / n_f

        c0 = ns_le_j + (Nb - nb_le_j)
        c1 = (Ns - ns_lt_j) + nb_lt_j
        c2 = (ns_le_j[None, :] - ns_lt_j[:, None]) + Nb - (
            nb_le_j[None, :] - nb_lt_j[:, None]
        )
        c3 = ns_le_j[:, None] + (Ns - ns_lt_j[None, :]) + (
            nb_le_j[None, :] - nb_lt_j[:, None]
        )

        L = jnp.stack(
            [
                jnp.broadcast_to(bce(c0)[:, None], (E, E)),
                jnp.broadcast_to(bce(c1)[:, None], (E, E)),
                bce(c2),
                bce(c3),
            ]
        )
        per_pair_min = jnp.min(L, axis=0)
        per_pair_case = jnp.argmin(L, axis=0)

        idxs = jnp.arange(E)
        valid = mask[:, None] & mask[None, :] & (idxs[:, None] < idxs[None, :])
        flat = jnp.argmin(jnp.where(valid, per_pair_min, jnp.inf))
        i = int(flat) // E
        j = int(flat) % E
        lower = np.float32(edges[i])
        upper = np.float32(edges[j])
        case = int(per_pair_case[i, j])

    # ---- L2: predicate on bf16 grid (original order) ----------------------
    lo_cmp = np.float32(BF(lower))
    up_cmp = np.float32(BF(upper))
    prm = np.zeros((P, 8), np.float32)
    prm[:, 0] = lo_cmp
    prm[:, 1] = up_cmp

    res3 = _run(
        f"pred{case}",
        [
            {"x": xb_all[c * CORE_N : c * CORE_N + DEV_N], "prm": prm}
            for c in CORE_IDS
        ],
    )

    out = np.empty(N, np.int32)
    for c in CORE_IDS:
        out[c * CORE_N : c * CORE_N + DEV_N] = (
            res3[c]["pred"].reshape(-1).astype(np.float32) != 0.0
        )
        # layout tail: exact on host
        tx = x[c * CORE_N + DEV_N : (c + 1) * CORE_N]
        if case == 0:
            tp = tx <= lower
        elif case == 1:
            tp = tx >= lower
        elif case == 2:
            tp = (tx >= lower) & (tx <= upper)
        else:
            tp = (tx <= lower) | (tx >= upper)
        out[c * CORE_N + DEV_N : (c + 1) * CORE_N] = tp.astype(np.int32)

    # repair events within bf16 rounding of the cuts (exact fp32 predicate)
    tol_lo = 6.0 / 128.0 * (abs(float(lower)) + 0.01)
    tol_up = 6.0 / 128.0 * (abs(float(upper)) + 0.01)
    rep = (np.abs(x - lower) <= tol_lo) | (np.abs(x - upper) <= tol_up)
    ri = np.flatnonzero(rep)
    rx = x[ri]
    if case == 0:
        rp = rx <= lower
    elif case == 1:
        rp = rx >= lower
    elif case == 2:
        rp = (rx >= lower) & (rx <= upper)
    else:
        rp = (rx <= lower) | (rx >= upper)
    out[ri] = rp.astype(np.int32)
    return out


# revision 9
# speedup vs baseline: 1.0388x; 1.0388x over previous
"""Trainium2 Bass kernel for nn_CutLayer (histogram_binning).

Two device launches over 8 cores (data-parallel on events):

L1 "counts": per-core class-compacted bf16 tile [128, F2] (rows 0-63 =
  signal events, rows 64-127 = background, pads = +BIG). 51 bf16-grid
  edge thresholds are counted by three engine paths running concurrently:
    - PE path (N_B edges): DVE plain tensor_scalar compare at 4x rate
      (bf16) into ping-pong buffers; PE reduces each compare tile with a
      one-hot stationary matmul into PSUM rows [2 per edge], accumulated
      across 512-column chunks; one final DVE reduce -> [2*N_B, 1].
    - DVE-accum path (N_A edges): fused tensor_scalar compare+accum (1x).
    - ACT path (N_C edges): Sign activation with bias strictly between
      bf16 grid points (no sign(0) ties) + accumulator.
  Host converts to exact fp32-semantics counts (le and lt) by correcting
  a small candidate set of events within a few bf16 ulps of each edge,
  then replicates the reference's pair search bit-exactly (eager CPU jax)
  to produce (lower, upper, case).

L2 "pred": case-specialized bf16 compare(s) on the original-order
  events; host flips the few events within bf16 rounding of the chosen
  cuts and handles the 512-event layout tail exactly.

Host work is O(N) numpy prep/fixup only: min/max, bf16 casts, class
compaction, candidate repair, tiny 51x51 pair search.
"""

from contextlib import ExitStack

import numpy as np
import ml_dtypes

import concourse.bass as bass
import concourse.mybir as mybir
from concourse.bass_utils import run_bass_kernel_spmd

N = 8_000_000
N_CORES = 8
CORE_N = N // N_CORES            # 1_000_000
P = 128
HP = 64                          # rows per class in the counts tile
N_BINS = 50
E = N_BINS + 1                   # 51 edges
EPS = 1e-7
BIG = np.float32(1.0e30)         # bf16-exact sentinel above every edge

# pred layout (original order)
FP = 7812
DEV_N = P * FP                   # 999_936 device events/core for pred

# counts engine split (sums to E)
N_B = 26                         # DVE compare -> PE matmul reduce
N_A = 7                          # DVE fused compare+accum
N_C = 18                         # ACT sign path
N_F = 17                         # of the N_B edges, pool-folded ones
NCB = 4                          # compare-buffer ping-pong depth
CH = 512                         # psum chunk columns

FP32 = mybir.dt.float32
BF16 = mybir.dt.bfloat16
I32 = mybir.dt.int32
AX = mybir.AxisListType
OP = mybir.AluOpType
ACT = mybir.ActivationFunctionType

CORE_IDS = list(range(N_CORES))
BF = ml_dtypes.bfloat16


# --------------------------------------------------------------------------
# Bass programs
# --------------------------------------------------------------------------

def _build_counts_v2(F2: int):
    nchunks = (F2 + CH - 1) // CH
    H2 = F2 // 2
    nhalf = (H2 + CH - 1) // CH
    MROWS = 2 * N_B
    # which B-edges are pool-folded (PE does half the matmuls on those)
    fold = [(i * N_F) // N_B > ((i - 1) * N_F) // N_B for i in range(N_B)]
    fidx = {}
    uidx = {}
    for i in range(N_B):
        if fold[i]:
            fidx[i] = len(fidx)
        else:
            uidx[i] = len(uidx)
    nc = bass.Bass()
    xt_in = nc.declare_dram_parameter("xt", [P, F2], BF16, isOutput=False)
    ed = nc.declare_dram_parameter("edges", [P, 2 * E], FP32, isOutput=False)
    oh_in = nc.declare_dram_parameter("oh", [P, N_B * MROWS], BF16,
                                      isOutput=False)
    acca_o = nc.declare_dram_parameter("acca", [P, N_A], FP32, isOutput=True)
    accc_o = nc.declare_dram_parameter("accc", [P, N_C], FP32, isOutput=True)
    accb_o = nc.declare_dram_parameter("accb", [MROWS, 1], FP32, isOutput=True)
    with ExitStack() as es:
        ec = es.enter_context
        xt = ec(nc.sbuf_tensor([P, F2], BF16))
        cb = [ec(nc.sbuf_tensor(f"cb{i}", [P, F2], BF16)) for i in range(NCB)]
        fb = [ec(nc.sbuf_tensor(f"fb{i}", [P, H2], BF16)) for i in range(2)]
        scra = ec(nc.sbuf_tensor([P, F2], BF16))
        scrc = ec(nc.sbuf_tensor([P, F2], BF16))
        edt = ec(nc.sbuf_tensor([P, 2 * E], FP32))
        oht = ec(nc.sbuf_tensor([P, N_B * MROWS], BF16))
        acca = ec(nc.sbuf_tensor([P, N_A], FP32))
        accc = ec(nc.sbuf_tensor([P, N_C], FP32))
        accb = ec(nc.sbuf_tensor([MROWS, 1], FP32))
        ps = ec(nc.psum_tensor([MROWS, CH], FP32))
        dse = ec(nc.semaphore("dse"))
        dsx0 = ec(nc.semaphore("dsx0"))
        dsx1 = ec(nc.semaphore("dsx1"))
        vprod = ec(nc.semaphore("vprod"))
        ucons = ec(nc.semaphore("ucons"))   # PE consumed unfolded edge
        pfold = ec(nc.semaphore("pfold"))   # pool folded edge f (also frees cb)
        fdone = ec(nc.semaphore("fdone"))   # PE consumed fold f
        adone = ec(nc.semaphore("adone"))
        cdone = ec(nc.semaphore("cdone"))
        bdone = ec(nc.semaphore("bdone"))
        dso = ec(nc.semaphore("dso"))
        block = ec(nc.Block())

        @block.sync
        def _(sync):
            sync.wait_ge(cdone, 1)
            sync.dma_start(accc_o[:], accc[:]).then_inc(dso, 16)
            sync.wait_ge(adone, 1)
            sync.dma_start(acca_o[:], acca[:]).then_inc(dso, 16)
            sync.wait_ge(bdone, 1)
            sync.dma_start(accb_o[:], accb[:]).then_inc(dso, 16)
            sync.wait_ge(dso, 48)

        @block.scalar
        def _(scalar):
            scalar.dma_start(edt[:], ed[:]).then_inc(dse, 16)
            scalar.dma_start(oht[:], oh_in[:]).then_inc(dse, 16)
            scalar.dma_start(xt[:, 0:H2], xt_in[:, 0:H2]).then_inc(dsx0, 16)
            scalar.wait_ge(dse, 32)
            scalar.wait_ge(dsx0, 16)
            scalar.wait_ge(dsx1, 16)
            for j in range(N_C):
                ins = scalar.activation(
                    scrc[:], xt[:], ACT.Sign, bias=edt[:, E + j : E + j + 1],
                    scale=1.0, accum_out=accc[:, j : j + 1],
                )
                if j == N_C - 1:
                    ins.then_inc(cdone, 1)

        @block.vector
        def _(vector):
            vector.wait_ge(dse, 32)
            vector.wait_ge(dsx0, 16)
            vector.wait_ge(dsx1, 16)
            na = 0
            last_a_ins = None
            for i in range(N_B):
                j = i - NCB
                if j >= 0:
                    if fold[j]:
                        vector.wait_ge(pfold, fidx[j] + 1)
                    else:
                        vector.wait_ge(ucons, uidx[j] + 1)
                vector.tensor_scalar(
                    cb[i % NCB][:], xt[:], edt[:, N_A + i : N_A + i + 1], None,
                    OP.is_le,
                ).then_inc(vprod, 1)
                if (i % 4 == 3) and na < N_A:
                    last_a_ins = vector.tensor_scalar(
                        scra[:], xt[:], edt[:, na : na + 1], 0.0,
                        OP.is_le, OP.add, accum_out=acca[:, na : na + 1],
                    )
                    na += 1
            while na < N_A:
                last_a_ins = vector.tensor_scalar(
                    scra[:], xt[:], edt[:, na : na + 1], 0.0,
                    OP.is_le, OP.add, accum_out=acca[:, na : na + 1],
                )
                na += 1
            if last_a_ins is not None:
                last_a_ins.then_inc(adone, 1)
            vector.wait_ge(ucons, len(uidx))
            vector.wait_ge(fdone, len(fidx))
            vector.tensor_reduce(
                accb[:, 0:1], ps[:, 0:CH], axis=AX.X, op=OP.add
            ).then_inc(bdone, 1)

        @block.gpsimd
        def _(gpsimd):
            gpsimd.dma_start(xt[:, H2:F2], xt_in[:, H2:F2]).then_inc(dsx1, 16)
            for i in range(N_B):
                if not fold[i]:
                    continue
                f = fidx[i]
                gpsimd.wait_ge(vprod, i + 1)
                if f >= 2:
                    gpsimd.wait_ge(fdone, f - 1)
                gpsimd.tensor_tensor(
                    fb[f % 2][:], cb[i % NCB][:, 0:H2], cb[i % NCB][:, H2:F2],
                    OP.add,
                ).then_inc(pfold, 1)

        @block.tensor
        def _(tensor):
            first = True
            for i in range(N_B):
                w = oht[:, i * MROWS : (i + 1) * MROWS]
                if fold[i]:
                    f = fidx[i]
                    tensor.wait_ge(pfold, f + 1)
                    src_t = fb[f % 2]
                    nck = nhalf
                    cap = H2
                else:
                    tensor.wait_ge(vprod, i + 1)
                    src_t = cb[i % NCB]
                    nck = nchunks
                    cap = F2
                for c in range(nck):
                    c0 = c * CH
                    c1 = min(cap, c0 + CH)
                    ins = tensor.matmul(
                        ps[:, 0 : c1 - c0], w, src_t[:, c0:c1],
                        start=first,
                        stop=(i == N_B - 1 and c == nck - 1),
                        skip_group_check=True,
                    )
                    if c > 0:
                        ins.ins.ldweights = False
                    first = False
                if fold[i]:
                    ins.then_inc(fdone, 1)
                else:
                    ins.then_inc(ucons, 1)
    return nc


def _build_pred(case: int):
    """Case-specialized predicate on bf16 events (original order):
    0: x <= lo ; 1: x >= lo ; 2: (x >= lo) & (x <= up) ;
    3: (x <= lo) | (x >= up)  (disjoint -> add)
    """
    nc = bass.Bass()
    x = nc.declare_dram_parameter("x", [DEV_N], BF16, isOutput=False)
    pr = nc.declare_dram_parameter("prm", [P, 8], FP32, isOutput=False)
    out = nc.declare_dram_parameter("pred", [DEV_N], BF16, isOutput=True)
    HF = FP // 2
    with (
        nc.sbuf_tensor([P, FP], BF16) as xt,
        nc.sbuf_tensor([P, FP], BF16) as t,
        nc.sbuf_tensor([P, FP], BF16) as s,
        nc.sbuf_tensor([P, FP], BF16) as pi,
        nc.sbuf_tensor([P, 8], FP32) as prm,
        nc.semaphore("d0") as d0,
        nc.semaphore("d1") as d1,
        nc.semaphore("csem") as csem,
        nc.semaphore("dso") as dso,
        nc.Block() as block,
    ):
        xv = x[:].rearrange("(p f) -> p f", p=P)
        ov = out[:].rearrange("(p f) -> p f", p=P)

        @block.sync
        def _(sync):
            sync.wait_ge(dso, 32)

        @block.scalar
        def _(scalar):
            scalar.dma_start(prm[:], pr[:]).then_inc(d0, 16)
            scalar.dma_start(xt[:, 0:HF], xv[:, 0:HF]).then_inc(d0, 16)
            scalar.wait_ge(csem, 1)
            scalar.dma_start(ov[:, 0:HF], pi[:, 0:HF]).then_inc(dso, 16)
            scalar.wait_ge(csem, 2)
            scalar.dma_start(ov[:, HF:FP], pi[:, HF:FP]).then_inc(dso, 16)

        @block.gpsimd
        def _(gpsimd):
            gpsimd.dma_start(xt[:, HF:FP], xv[:, HF:FP]).then_inc(d1, 16)

        @block.vector
        def _(vector):
            lo = prm[:, 0:1]
            up = prm[:, 1:2]
            vector.wait_ge(d0, 32)
            for h in (0, 1):
                if h == 1:
                    vector.wait_ge(d1, 16)
                sl = slice(0, HF) if h == 0 else slice(HF, FP)
                if case == 0:
                    vector.tensor_scalar(
                        pi[:, sl], xt[:, sl], lo, None, OP.is_le
                    ).then_inc(csem, 1)
                elif case == 1:
                    vector.tensor_scalar(
                        pi[:, sl], xt[:, sl], lo, None, OP.is_ge
                    ).then_inc(csem, 1)
                elif case == 2:
                    vector.tensor_scalar(t[:, sl], xt[:, sl], up, None,
                                         OP.is_le)
                    vector.tensor_scalar(s[:, sl], xt[:, sl], lo, None,
                                         OP.is_ge)
                    vector.tensor_tensor(
                        pi[:, sl], s[:, sl], t[:, sl], OP.mult
                    ).then_inc(csem, 1)
                else:
                    vector.tensor_scalar(t[:, sl], xt[:, sl], up, None,
                                         OP.is_ge)
                    vector.tensor_scalar(s[:, sl], xt[:, sl], lo, None,
                                         OP.is_le)
                    vector.tensor_tensor(
                        pi[:, sl], s[:, sl], t[:, sl], OP.add
                    ).then_inc(csem, 1)
    return nc


_PROGRAMS: dict = {}


def _prog(name, *args):
    key = (name, args)
    if key not in _PROGRAMS:
        if name == "counts":
            _PROGRAMS[key] = _build_counts_v2(*args)
        else:
            _PROGRAMS[key] = _build_pred(int(name[4:]))
    return _PROGRAMS[key]


LAST_EXEC_NS: list = []
_CACHE_SET = False


def _enable_jit_cache():
    global _CACHE_SET
    if _CACHE_SET:
        return
    _CACHE_SET = True
    try:
        import jax

        jax.config.update("jax_compilation_cache_dir", "/tmp/jax_bass_cache")
        jax.config.update("jax_persistent_cache_min_compile_time_secs", 1.0)
        jax.config.update("jax_persistent_cache_min_entry_size_bytes", 0)
    except Exception:
        pass


def _run(name, in_maps, *args):
    import os

    _enable_jit_cache()
    trace = bool(int(os.environ.get("BASS_KERNEL_PROFILE", "0")))
    r = run_bass_kernel_spmd(_prog(name, *args), in_maps, CORE_IDS, trace=trace)
    if trace:
        LAST_EXEC_NS.append((name, r.exec_time_ns, r.mean_exec_time_ns))
    return r.results


# --------------------------------------------------------------------------
# Host orchestration
# --------------------------------------------------------------------------

def _ulp_quarter(e64):
    """0.25 * (lower bound of the bf16 ulp at e), elementwise, float64."""
    a = np.abs(e64)
    a = np.where(a < 1e-30, 1e-30, a)
    return 0.25 * np.exp2(np.floor(np.log2(a)) - 8.0)


def kernel(inputs: np.ndarray, targets: np.ndarray) -> np.ndarray:
    import jax
    import jax.numpy as jnp

    x = np.ascontiguousarray(inputs[:, 0]).astype(np.float32, copy=False)
    y = np.asarray(targets)
    sig = y == 1

    LAST_EXEC_NS.clear()

    # ---- edges (host, bit-exact with the reference) -----------------------
    gmin = np.float32(x.min())
    gmax = np.float32(x.max())
    cpu = jax.devices("cpu")[0]
    with jax.default_device(cpu):
        edges = np.asarray(jnp.linspace(jnp.float32(gmin), jnp.float32(gmax), E))
    e_cmp = edges.astype(BF).astype(np.float32)          # bf16-grid thresholds
    e_act64 = e_cmp.astype(np.float64) + _ulp_quarter(e_cmp.astype(np.float64))
    e_act = e_act64.astype(np.float32)                   # strictly between grid pts

    # ---- class-compacted bf16 tiles --------------------------------------
    xb_all = x.astype(BF)
    xs_sig = xb_all[sig]
    xs_bg = xb_all[~sig]
    Ns_i, Nb_i = xs_sig.size, xs_bg.size
    cap_rows = N_CORES * HP
    F2 = (max(Ns_i, Nb_i) + cap_rows - 1) // cap_rows
    if F2 % 2:
        F2 += 1
    big_bf = BF(BIG)
    sig_pad = np.full(cap_rows * F2, big_bf, dtype=BF)
    sig_pad[:Ns_i] = xs_sig
    bg_pad = np.full(cap_rows * F2, big_bf, dtype=BF)
    bg_pad[:Nb_i] = xs_bg
    sig_tiles = sig_pad.reshape(N_CORES, HP, F2)
    bg_tiles = bg_pad.reshape(N_CORES, HP, F2)

    # edge -> slot assignment: A slots 0..N_A-1, B slots N_A..N_A+N_B-1 use
    # e_cmp; C slots use -e_act. Keep natural order (all e_act distinct from
    # e_cmp by construction; _ulp_quarter guards tiny magnitudes).
    ed_in = np.zeros((P, 2 * E), np.float32)
    ed_in[:, :E] = np.concatenate([e_cmp[:N_A], e_cmp[N_A : N_A + N_B],
                                   np.zeros(E - N_A - N_B, np.float32)])
    ed_in[:, E : E + N_C] = -e_act[N_A + N_B :]

    MROWS = 2 * N_B
    oh = np.zeros((P, N_B * MROWS), BF)
    for i in range(N_B):
        oh[0:HP, i * MROWS + 2 * i] = 1        # signal rows -> psum row 2i
        oh[HP:P, i * MROWS + 2 * i + 1] = 1    # background rows -> 2i+1

    res = _run(
        "counts",
        [
            {
                "xt": np.ascontiguousarray(
                    np.concatenate([sig_tiles[c], bg_tiles[c]], axis=0)
                ),
                "edges": ed_in,
                "oh": oh,
            }
            for c in CORE_IDS
        ],
        F2,
    )

    # ---- decode device counts (counts of xb <= e_cmp[k], per class) ------
    d_sig = np.zeros(E, np.float64)
    d_bg = np.zeros(E, np.float64)
    TOT_HALF = N_CORES * HP * F2
    for r in res:
        a = r["acca"].astype(np.float64)      # [P, N_A]
        c = r["accc"].astype(np.float64)      # [P, N_C]
        b = r["accb"].astype(np.float64)      # [MROWS, 1]
        d_sig[0:N_A] += a[0:HP].sum(axis=0)
        d_bg[0:N_A] += a[HP:P].sum(axis=0)
        d_sig[N_A : N_A + N_B] += b[0::2, 0]
        d_bg[N_A : N_A + N_B] += b[1::2, 0]
        # ACT: S = sum sign(x - e_act); le = (TOT - S)/2 per class half
        d_sig[N_A + N_B :] -= c[0:HP].sum(axis=0) / 2.0
        d_bg[N_A + N_B :] -= c[HP:P].sum(axis=0) / 2.0
    d_sig[N_A + N_B :] += TOT_HALF / 2.0
    d_bg[N_A + N_B :] += TOT_HALF / 2.0
    # pads count as "greater" in every path: subtract nothing for le counts.

    # ---- host repair: exact le/lt counts under fp32 semantics ------------
    h64 = (np.float64(gmax) - np.float64(gmin)) / N_BINS
    inv_h = np.float32(1.0 / h64) if h64 != 0 else np.float32(0.0)
    u = (x - gmin) * inv_h
    k0 = np.rint(u).astype(np.int64)
    tol = (6.0 / 128.0) * (np.abs(edges.astype(np.float64)) + 0.01)
    cand_mask = np.zeros(N, bool)
    for dk in (-1, 0, 1):
        kk = np.clip(k0 + dk, 0, E - 1)
        cand_mask |= np.abs(x.astype(np.float64) - edges[kk]) <= tol[kk]
    ci = np.flatnonzero(cand_mask)
    cx = x[ci]
    cxb = xb_all[ci].astype(np.float32)
    csig = sig[ci]
    ck0 = k0[ci]

    corr_le_sig = np.zeros(E, np.float64)
    corr_le_bg = np.zeros(E, np.float64)
    corr_lt_sig = np.zeros(E, np.float64)
    corr_lt_bg = np.zeros(E, np.float64)
    for dk in (-1, 0, 1):
        kr = ck0 + dk
        use = (kr >= 0) & (kr < E)
        kku = kr[use]
        dev = (cxb[use] <= e_cmp[kku]).astype(np.float64)
        t_le = (cx[use] <= edges[kku]).astype(np.float64)
        t_lt = (cx[use] < edges[kku]).astype(np.float64)
        s_u = csig[use]
        np.add.at(corr_le_sig, kku[s_u], (t_le - dev)[s_u])
        np.add.at(corr_le_bg, kku[~s_u], (t_le - dev)[~s_u])
        np.add.at(corr_lt_sig, kku[s_u], (t_lt - dev)[s_u])
        np.add.at(corr_lt_bg, kku[~s_u], (t_lt - dev)[~s_u])

    ns_le = (d_sig + corr_le_sig).astype(np.float32)
    nb_le = (d_bg + corr_le_bg).astype(np.float32)
    ns_lt = (d_sig + corr_lt_sig).astype(np.float32)
    nb_lt = (d_bg + corr_lt_bg).astype(np.float32)

    # ---- replicate the reference's pair search bit-exactly ----------------
    with jax.default_device(cpu):
        ns_le_j = jnp.asarray(ns_le)
        ns_lt_j = jnp.asarray(ns_lt)
        nb_le_j = jnp.asarray(nb_le)
        nb_lt_j = jnp.asarray(nb_lt)
        n_f = jnp.float32(N)
        Ns = ns_le_j[-1]
        Nb = n_f - Ns

        hist0 = nb_le_j[1:] - nb_lt_j[:-1]
        hist1 = ns_le_j[1:] - ns_lt_j[:-1]

        gt0 = hist0 > hist1
        cand0 = jnp.logical_xor(gt0[:-1], gt0[1:]) & (hist0[:-1] > 0)
        gt1 = hist1 > hist0
        cand1 = jnp.logical_xor(gt1[:-1], gt1[1:]) & (hist1[:-1] > 0)
        mask = jnp.zeros((E,), bool).at[1:N_BINS].set(cand0 | cand1)
        cnt = jnp.sum(mask)
        mask = mask.at[-1].set(mask[-1] | (cnt == 1))

        a_c = -jnp.log1p(jnp.float32(-EPS))
        b_c = -jnp.log(jnp.float32(EPS))

        def bce(correct):
            return ((n_f - correct) * b_c + correct * a_c) / n_f

        c0 = ns_le_j + (Nb - nb_le_j)
        c1 = (Ns - ns_lt_j) + nb_lt_j
        c2 = (ns_le_j[None, :] - ns_lt_j[:, None]) + Nb - (
            nb_le_j[None, :] - nb_lt_j[:, None]
        )
        c3 = ns_le_j[:, None] + (Ns - ns_lt_j[None, :]) + (
            nb_le_j[None, :] - nb_lt_j[:, None]
        )

        L = jnp.stack(
            [
                jnp.broadcast_to(bce(c0)[:, None], (E, E)),
                jnp.broadcast_to(bce(c1)[:, None], (E, E)),
                bce(c2),
                bce(c3),
            ]
        )
        per_pair_min = jnp.min(L, axis=0)
        per_pair_case = jnp.argmin(L, axis=0)

        idxs = jnp.arange(E)
        valid = mask[:, None] & mask[None, :] & (idxs[:, None] < idxs[None, :])
        flat = jnp.argmin(jnp.where(valid, per_pair_min, jnp.inf))
        i = int(flat) // E
        j = int(flat) % E
        lower = np.float32(edges[i])
        upper = np.float32(edges[j])
        case = int(per_pair_case[i, j])

    # ---- L2: predicate on bf16 grid (original order) ----------------------
    lo_cmp = np.float32(BF(lower))
    up_cmp = np.float32(BF(upper))
    prm = np.zeros((P, 8), np.float32)
    prm[:, 0] = lo_cmp
    prm[:, 1] = up_cmp

    res3 = _run(
        f"pred{case}",
        [
            {"x": xb_all[c * CORE_N : c * CORE_N + DEV_N], "prm": prm}
            for c in CORE_IDS
        ],
    )

    out = np.empty(N, np.int32)
    for c in CORE_IDS:
        out[c * CORE_N : c * CORE_N + DEV_N] = (
            res3[c]["pred"].reshape(-1).astype(np.float32) != 0.0
        )
        # layout tail: exact on host
        tx = x[c * CORE_N + DEV_N : (c + 1) * CORE_N]
        if case == 0:
            tp = tx <= lower
        elif case == 1:
            tp = tx >= lower
        elif case == 2:
            tp = (tx >= lower) & (tx <= upper)
        else:
            tp = (tx <= lower) | (tx >= upper)
        out[c * CORE_N + DEV_N : (c + 1) * CORE_N] = tp.astype(np.int32)

    # repair events within bf16 rounding of the cuts (exact fp32 predicate)
    tol_lo = 6.0 / 128.0 * (abs(float(lower)) + 0.01)
    tol_up = 6.0 / 128.0 * (abs(float(upper)) + 0.01)
    rep = (np.abs(x - lower) <= tol_lo) | (np.abs(x - upper) <= tol_up)
    ri = np.flatnonzero(rep)
    rx = x[ri]
    if case == 0:
        rp = rx <= lower
    elif case == 1:
        rp = rx >= lower
    elif case == 2:
        rp = (rx >= lower) & (rx <= upper)
    else:
        rp = (rx <= lower) | (rx >= upper)
    out[ri] = rp.astype(np.int32)
    return out


# revision 11
# speedup vs baseline: 1.3077x; 1.2588x over previous
"""Trainium2 Bass kernel for nn_CutLayer (histogram_binning).

Two device launches over 8 cores (data-parallel on events):

L1 "counts": per-core class-compacted bf16 tile [128, F2] (rows 0-63 =
  signal events, rows 64-127 = background, pads = +BIG). 51 bf16-grid
  edge thresholds are counted by three engine paths running concurrently:
    - PE path (N_B edges): DVE plain tensor_scalar compare at 4x rate
      (bf16) into ping-pong buffers; PE reduces each compare tile with a
      one-hot stationary matmul into PSUM rows [2 per edge], accumulated
      across 512-column chunks; one final DVE reduce -> [2*N_B, 1].
    - DVE-accum path (N_A edges): fused tensor_scalar compare+accum (1x).
    - ACT path (N_C edges): Sign activation with bias strictly between
      bf16 grid points (no sign(0) ties) + accumulator.
  Host converts to exact fp32-semantics counts (le and lt) by correcting
  a small candidate set of events within a few bf16 ulps of each edge,
  then replicates the reference's pair search bit-exactly (eager CPU jax)
  to produce (lower, upper, case).

L2 "pred": case-specialized bf16 compare(s) on the original-order
  events; host flips the few events within bf16 rounding of the chosen
  cuts and handles the 512-event layout tail exactly.

Host work is O(N) numpy prep/fixup only: min/max, bf16 casts, class
compaction, candidate repair, tiny 51x51 pair search.
"""

from contextlib import ExitStack

import numpy as np
import ml_dtypes

import concourse.bass as bass
import concourse.mybir as mybir
from concourse.bass_utils import run_bass_kernel_spmd

N = 8_000_000
N_CORES = 8
CORE_N = N // N_CORES            # 1_000_000
P = 128
HP = 64                          # rows per class in the counts tile
N_BINS = 50
E = N_BINS + 1                   # 51 edges
EPS = 1e-7
BIG = np.float32(1.0e30)         # bf16-exact sentinel above every edge

# pred layout (original order)
FP = 7812
DEV_N = P * FP                   # 999_936 device events/core for pred

# counts engine split (sums to E)
N_B = 22                         # DVE compare -> PE matmul reduce
N_A = 10                         # DVE fused compare+accum
N_C = 19                         # ACT sign path
N_F = 0                          # of the N_B edges, pool-folded ones
NCB = 4                          # compare-buffer ping-pong depth
CH = 512                         # psum chunk columns

FP32 = mybir.dt.float32
BF16 = mybir.dt.bfloat16
I32 = mybir.dt.int32
AX = mybir.AxisListType
OP = mybir.AluOpType
ACT = mybir.ActivationFunctionType

CORE_IDS = list(range(N_CORES))
BF = ml_dtypes.bfloat16


# --------------------------------------------------------------------------
# Bass programs
# --------------------------------------------------------------------------

def _build_counts_v2(F2: int):
    nchunks = (F2 + CH - 1) // CH
    H2 = F2 // 2
    nhalf = (H2 + CH - 1) // CH
    MROWS = 2 * N_B
    # which B-edges are pool-folded (PE does half the matmuls on those)
    fold = [(i * N_F) // N_B > ((i - 1) * N_F) // N_B for i in range(N_B)]
    fidx = {}
    uidx = {}
    for i in range(N_B):
        if fold[i]:
            fidx[i] = len(fidx)
        else:
            uidx[i] = len(uidx)
    nc = bass.Bass()
    xt_in = nc.declare_dram_parameter("xt", [P, F2], BF16, isOutput=False)
    ed = nc.declare_dram_parameter("edges", [P, 2 * E], FP32, isOutput=False)
    oh_in = nc.declare_dram_parameter("oh", [P, N_B * MROWS], BF16,
                                      isOutput=False)
    acca_o = nc.declare_dram_parameter("acca", [P, N_A], FP32, isOutput=True)
    accc_o = nc.declare_dram_parameter("accc", [P, N_C], FP32, isOutput=True)
    accb_o = nc.declare_dram_parameter("accb", [MROWS, 1], FP32, isOutput=True)
    with ExitStack() as es:
        ec = es.enter_context
        xt = ec(nc.sbuf_tensor([P, F2], BF16))
        cb = [ec(nc.sbuf_tensor(f"cb{i}", [P, F2], BF16)) for i in range(NCB)]
        fb = [ec(nc.sbuf_tensor(f"fb{i}", [P, H2], BF16)) for i in range(2)]
        scra = ec(nc.sbuf_tensor([P, F2], BF16))
        scrc = ec(nc.sbuf_tensor([P, F2], BF16))
        edt = ec(nc.sbuf_tensor([P, 2 * E], FP32))
        oht = ec(nc.sbuf_tensor([P, N_B * MROWS], BF16))
        acca = ec(nc.sbuf_tensor([P, N_A], FP32))
        accc = ec(nc.sbuf_tensor([P, N_C], FP32))
        accb = ec(nc.sbuf_tensor([MROWS, 1], FP32))
        ps = ec(nc.psum_tensor([MROWS, CH], FP32))
        dse = ec(nc.semaphore("dse"))
        dsx0 = ec(nc.semaphore("dsx0"))
        dsx1 = ec(nc.semaphore("dsx1"))
        dsx2 = ec(nc.semaphore("dsx2"))
        vprod = ec(nc.semaphore("vprod"))
        ucons = ec(nc.semaphore("ucons"))   # PE consumed unfolded edge
        pfold = ec(nc.semaphore("pfold"))   # pool folded edge f (also frees cb)
        fdone = ec(nc.semaphore("fdone"))   # PE consumed fold f
        adone = ec(nc.semaphore("adone"))
        cdone = ec(nc.semaphore("cdone"))
        bdone = ec(nc.semaphore("bdone"))
        dso = ec(nc.semaphore("dso"))
        block = ec(nc.Block())

        S1 = (int(0.60 * F2) // 2) * 2
        S2 = (int(0.85 * F2) // 2) * 2

        @block.sync
        def _(sync):
            sync.dma_start(xt[:, S1:S2], xt_in[:, S1:S2]).then_inc(dsx1, 16)
            sync.wait_ge(cdone, 1)
            sync.dma_start(accc_o[:], accc[:]).then_inc(dso, 16)
            sync.wait_ge(adone, 1)
            sync.dma_start(acca_o[:], acca[:]).then_inc(dso, 16)
            sync.wait_ge(bdone, 1)
            sync.dma_start(accb_o[:], accb[:]).then_inc(dso, 16)
            sync.wait_ge(dso, 48)

        @block.scalar
        def _(scalar):
            scalar.dma_start(edt[:], ed[:]).then_inc(dse, 16)
            scalar.dma_start(oht[:], oh_in[:]).then_inc(dse, 16)
            scalar.dma_start(xt[:, 0:S1], xt_in[:, 0:S1]).then_inc(dsx0, 16)
            scalar.wait_ge(dse, 32)
            scalar.wait_ge(dsx0, 16)
            scalar.wait_ge(dsx1, 16)
            scalar.wait_ge(dsx2, 16)
            for j in range(N_C):
                ins = scalar.activation(
                    scrc[:], xt[:], ACT.Sign, bias=edt[:, E + j : E + j + 1],
                    scale=1.0, accum_out=accc[:, j : j + 1],
                )
                if j == N_C - 1:
                    ins.then_inc(cdone, 1)

        @block.vector
        def _(vector):
            vector.wait_ge(dse, 32)
            vector.wait_ge(dsx0, 16)
            vector.wait_ge(dsx1, 16)
            vector.wait_ge(dsx2, 16)
            na = 0
            last_a_ins = None
            for i in range(N_B):
                j = i - NCB
                if j >= 0:
                    if fold[j]:
                        vector.wait_ge(pfold, fidx[j] + 1)
                    else:
                        vector.wait_ge(ucons, uidx[j] + 1)
                vector.tensor_scalar(
                    cb[i % NCB][:], xt[:], edt[:, N_A + i : N_A + i + 1], None,
                    OP.is_le,
                ).then_inc(vprod, 1)
                if (i % 4 == 3) and na < N_A:
                    last_a_ins = vector.tensor_scalar(
                        scra[:], xt[:], edt[:, na : na + 1], 0.0,
                        OP.is_le, OP.add, accum_out=acca[:, na : na + 1],
                    )
                    na += 1
            while na < N_A:
                last_a_ins = vector.tensor_scalar(
                    scra[:], xt[:], edt[:, na : na + 1], 0.0,
                    OP.is_le, OP.add, accum_out=acca[:, na : na + 1],
                )
                na += 1
            if last_a_ins is not None:
                last_a_ins.then_inc(adone, 1)
            vector.wait_ge(ucons, len(uidx))
            vector.wait_ge(fdone, len(fidx))
            vector.tensor_reduce(
                accb[:, 0:1], ps[:, 0:CH], axis=AX.X, op=OP.add
            ).then_inc(bdone, 1)

        @block.gpsimd
        def _(gpsimd):
            gpsimd.dma_start(xt[:, S2:F2], xt_in[:, S2:F2]).then_inc(dsx2, 16)
            for i in range(N_B):
                if not fold[i]:
                    continue
                f = fidx[i]
                gpsimd.wait_ge(vprod, i + 1)
                if f >= 2:
                    gpsimd.wait_ge(fdone, f - 1)
                gpsimd.tensor_tensor(
                    fb[f % 2][:], cb[i % NCB][:, 0:H2], cb[i % NCB][:, H2:F2],
                    OP.add,
                ).then_inc(pfold, 1)

        @block.tensor
        def _(tensor):
            first = True
            for i in range(N_B):
                w = oht[:, i * MROWS : (i + 1) * MROWS]
                if fold[i]:
                    f = fidx[i]
                    tensor.wait_ge(pfold, f + 1)
                    src_t = fb[f % 2]
                    nck = nhalf
                    cap = H2
                else:
                    tensor.wait_ge(vprod, i + 1)
                    src_t = cb[i % NCB]
                    nck = nchunks
                    cap = F2
                for c in range(nck):
                    c0 = c * CH
                    c1 = min(cap, c0 + CH)
                    ins = tensor.matmul(
                        ps[:, 0 : c1 - c0], w, src_t[:, c0:c1],
                        start=first,
                        stop=(i == N_B - 1 and c == nck - 1),
                        skip_group_check=True,
                    )
                    if c > 0:
                        ins.ins.ldweights = False
                    first = False
                if fold[i]:
                    ins.then_inc(fdone, 1)
                else:
                    ins.then_inc(ucons, 1)
    return nc


def _build_pred(case: int):
    """Case-specialized predicate on bf16 events (original order):
    0: x <= lo ; 1: x >= lo ; 2: (x >= lo) & (x <= up) ;
    3: (x <= lo) | (x >= up)  (disjoint -> add)
    """
    nc = bass.Bass()
    x = nc.declare_dram_parameter("x", [DEV_N], BF16, isOutput=False)
    pr = nc.declare_dram_parameter("prm", [P, 8], FP32, isOutput=False)
    out = nc.declare_dram_parameter("pred", [DEV_N], BF16, isOutput=True)
    HF = FP // 2
    with (
        nc.sbuf_tensor([P, FP], BF16) as xt,
        nc.sbuf_tensor([P, FP], BF16) as t,
        nc.sbuf_tensor([P, FP], BF16) as s,
        nc.sbuf_tensor([P, FP], BF16) as pi,
        nc.sbuf_tensor([P, 8], FP32) as prm,
        nc.semaphore("d0") as d0,
        nc.semaphore("d1") as d1,
        nc.semaphore("csem") as csem,
        nc.semaphore("dso") as dso,
        nc.Block() as block,
    ):
        xv = x[:].rearrange("(p f) -> p f", p=P)
        ov = out[:].rearrange("(p f) -> p f", p=P)

        @block.sync
        def _(sync):
            sync.dma_start(prm[:], pr[:]).then_inc(d0, 16)
            sync.dma_start(xt[:, HF:FP], xv[:, HF:FP]).then_inc(d1, 16)
            sync.wait_ge(csem, 1)
            sync.dma_start(ov[:, 0:HF], pi[:, 0:HF]).then_inc(dso, 16)
            sync.wait_ge(dso, 32)

        @block.scalar
        def _(scalar):
            scalar.dma_start(xt[:, 0:HF], xv[:, 0:HF]).then_inc(d0, 16)
            scalar.wait_ge(csem, 2)
            scalar.dma_start(ov[:, HF:FP], pi[:, HF:FP]).then_inc(dso, 16)

        @block.vector
        def _(vector):
            lo = prm[:, 0:1]
            up = prm[:, 1:2]
            vector.wait_ge(d0, 32)
            for h in (0, 1):
                if h == 1:
                    vector.wait_ge(d1, 16)
                sl = slice(0, HF) if h == 0 else slice(HF, FP)
                if case == 0:
                    vector.tensor_scalar(
                        pi[:, sl], xt[:, sl], lo, None, OP.is_le
                    ).then_inc(csem, 1)
                elif case == 1:
                    vector.tensor_scalar(
                        pi[:, sl], xt[:, sl], lo, None, OP.is_ge
                    ).then_inc(csem, 1)
                elif case == 2:
                    vector.tensor_scalar(t[:, sl], xt[:, sl], up, None,
                                         OP.is_le)
                    vector.tensor_scalar(s[:, sl], xt[:, sl], lo, None,
                                         OP.is_ge)
                    vector.tensor_tensor(
                        pi[:, sl], s[:, sl], t[:, sl], OP.mult
                    ).then_inc(csem, 1)
                else:
                    vector.tensor_scalar(t[:, sl], xt[:, sl], up, None,
                                         OP.is_ge)
                    vector.tensor_scalar(s[:, sl], xt[:, sl], lo, None,
                                         OP.is_le)
                    vector.tensor_tensor(
                        pi[:, sl], s[:, sl], t[:, sl], OP.add
                    ).then_inc(csem, 1)
    return nc


_PROGRAMS: dict = {}


def _prog(name, *args):
    key = (name, args)
    if key not in _PROGRAMS:
        if name == "counts":
            _PROGRAMS[key] = _build_counts_v2(*args)
        else:
            _PROGRAMS[key] = _build_pred(int(name[4:]))
    return _PROGRAMS[key]


LAST_EXEC_NS: list = []
_CACHE_SET = False


def _enable_jit_cache():
    global _CACHE_SET
    if _CACHE_SET:
        return
    _CACHE_SET = True
    try:
        import jax

        jax.config.update("jax_compilation_cache_dir", "/tmp/jax_bass_cache")
        jax.config.update("jax_persistent_cache_min_compile_time_secs", 1.0)
        jax.config.update("jax_persistent_cache_min_entry_size_bytes", 0)
    except Exception:
        pass


def _run(name, in_maps, *args):
    import os

    _enable_jit_cache()
    trace = bool(int(os.environ.get("BASS_KERNEL_PROFILE", "0")))
    r = run_bass_kernel_spmd(_prog(name, *args), in_maps, CORE_IDS, trace=trace)
    if trace:
        LAST_EXEC_NS.append((name, r.exec_time_ns, r.mean_exec_time_ns))
    return r.results


# --------------------------------------------------------------------------
# Host orchestration
# --------------------------------------------------------------------------

def _ulp_quarter(e64):
    """0.25 * (lower bound of the bf16 ulp at e), elementwise, float64."""
    a = np.abs(e64)
    a = np.where(a < 1e-30, 1e-30, a)
    return 0.25 * np.exp2(np.floor(np.log2(a)) - 8.0)


def kernel(inputs: np.ndarray, targets: np.ndarray) -> np.ndarray:
    import jax
    import jax.numpy as jnp

    x = np.ascontiguousarray(inputs[:, 0]).astype(np.float32, copy=False)
    y = np.asarray(targets)
    sig = y == 1

    LAST_EXEC_NS.clear()

    # ---- edges (host, bit-exact with the reference) -----------------------
    gmin = np.float32(x.min())
    gmax = np.float32(x.max())
    cpu = jax.devices("cpu")[0]
    with jax.default_device(cpu):
        edges = np.asarray(jnp.linspace(jnp.float32(gmin), jnp.float32(gmax), E))
    e_cmp = edges.astype(BF).astype(np.float32)          # bf16-grid thresholds
    e_act64 = e_cmp.astype(np.float64) + _ulp_quarter(e_cmp.astype(np.float64))
    e_act = e_act64.astype(np.float32)                   # strictly between grid pts

    # ---- class-compacted bf16 tiles --------------------------------------
    xb_all = x.astype(BF)
    xs_sig = xb_all[sig]
    xs_bg = xb_all[~sig]
    Ns_i, Nb_i = xs_sig.size, xs_bg.size
    cap_rows = N_CORES * HP
    F2 = (max(Ns_i, Nb_i) + cap_rows - 1) // cap_rows
    if F2 % 2:
        F2 += 1
    big_bf = BF(BIG)
    sig_pad = np.full(cap_rows * F2, big_bf, dtype=BF)
    sig_pad[:Ns_i] = xs_sig
    bg_pad = np.full(cap_rows * F2, big_bf, dtype=BF)
    bg_pad[:Nb_i] = xs_bg
    sig_tiles = sig_pad.reshape(N_CORES, HP, F2)
    bg_tiles = bg_pad.reshape(N_CORES, HP, F2)

    # edge -> slot assignment: A slots 0..N_A-1, B slots N_A..N_A+N_B-1 use
    # e_cmp; C slots use -e_act. Keep natural order (all e_act distinct from
    # e_cmp by construction; _ulp_quarter guards tiny magnitudes).
    ed_in = np.zeros((P, 2 * E), np.float32)
    ed_in[:, :E] = np.concatenate([e_cmp[:N_A], e_cmp[N_A : N_A + N_B],
                                   np.zeros(E - N_A - N_B, np.float32)])
    ed_in[:, E : E + N_C] = -e_act[N_A + N_B :]

    MROWS = 2 * N_B
    oh = np.zeros((P, N_B * MROWS), BF)
    for i in range(N_B):
        oh[0:HP, i * MROWS + 2 * i] = 1        # signal rows -> psum row 2i
        oh[HP:P, i * MROWS + 2 * i + 1] = 1    # background rows -> 2i+1

    res = _run(
        "counts",
        [
            {
                "xt": np.ascontiguousarray(
                    np.concatenate([sig_tiles[c], bg_tiles[c]], axis=0)
                ),
                "edges": ed_in,
                "oh": oh,
            }
            for c in CORE_IDS
        ],
        F2,
    )

    # ---- decode device counts (counts of xb <= e_cmp[k], per class) ------
    d_sig = np.zeros(E, np.float64)
    d_bg = np.zeros(E, np.float64)
    TOT_HALF = N_CORES * HP * F2
    for r in res:
        a = r["acca"].astype(np.float64)      # [P, N_A]
        c = r["accc"].astype(np.float64)      # [P, N_C]
        b = r["accb"].astype(np.float64)      # [MROWS, 1]
        d_sig[0:N_A] += a[0:HP].sum(axis=0)
        d_bg[0:N_A] += a[HP:P].sum(axis=0)
        d_sig[N_A : N_A + N_B] += b[0::2, 0]
        d_bg[N_A : N_A + N_B] += b[1::2, 0]
        # ACT: S = sum sign(x - e_act); le = (TOT - S)/2 per class half
        d_sig[N_A + N_B :] -= c[0:HP].sum(axis=0) / 2.0
        d_bg[N_A + N_B :] -= c[HP:P].sum(axis=0) / 2.0
    d_sig[N_A + N_B :] += TOT_HALF / 2.0
    d_bg[N_A + N_B :] += TOT_HALF / 2.0
    # pads count as "greater" in every path: subtract nothing for le counts.

    # ---- host repair: exact le/lt counts under fp32 semantics ------------
    h64 = (np.float64(gmax) - np.float64(gmin)) / N_BINS
    inv_h = np.float32(1.0 / h64) if h64 != 0 else np.float32(0.0)
    u = (x - gmin) * inv_h
    k0 = np.rint(u).astype(np.int64)
    tol = (6.0 / 128.0) * (np.abs(edges.astype(np.float64)) + 0.01)
    cand_mask = np.zeros(N, bool)
    for dk in (-1, 0, 1):
        kk = np.clip(k0 + dk, 0, E - 1)
        cand_mask |= np.abs(x.astype(np.float64) - edges[kk]) <= tol[kk]
    ci = np.flatnonzero(cand_mask)
    cx = x[ci]
    cxb = xb_all[ci].astype(np.float32)
    csig = sig[ci]
    ck0 = k0[ci]

    corr_le_sig = np.zeros(E, np.float64)
    corr_le_bg = np.zeros(E, np.float64)
    corr_lt_sig = np.zeros(E, np.float64)
    corr_lt_bg = np.zeros(E, np.float64)
    for dk in (-1, 0, 1):
        kr = ck0 + dk
        use = (kr >= 0) & (kr < E)
        kku = kr[use]
        dev = (cxb[use] <= e_cmp[kku]).astype(np.float64)
        t_le = (cx[use] <= edges[kku]).astype(np.float64)
        t_lt = (cx[use] < edges[kku]).astype(np.float64)
        s_u = csig[use]
        np.add.at(corr_le_sig, kku[s_u], (t_le - dev)[s_u])
        np.add.at(corr_le_bg, kku[~s_u], (t_le - dev)[~s_u])
        np.add.at(corr_lt_sig, kku[s_u], (t_lt - dev)[s_u])
        np.add.at(corr_lt_bg, kku[~s_u], (t_lt - dev)[~s_u])

    ns_le = (d_sig + corr_le_sig).astype(np.float32)
    nb_le = (d_bg + corr_le_bg).astype(np.float32)
    ns_lt = (d_sig + corr_lt_sig).astype(np.float32)
    nb_lt = (d_bg + corr_lt_bg).astype(np.float32)

    # ---- replicate the reference's pair search bit-exactly ----------------
    with jax.default_device(cpu):
        ns_le_j = jnp.asarray(ns_le)
        ns_lt_j = jnp.asarray(ns_lt)
        nb_le_j = jnp.asarray(nb_le)
        nb_lt_j = jnp.asarray(nb_lt)
        n_f = jnp.float32(N)
        Ns = ns_le_j[-1]
        Nb = n_f - Ns

        hist0 = nb_le_j[1:] - nb_lt_j[:-1]
        hist1 = ns_le_j[1:] - ns_lt_j[:-1]

        gt0 = hist0 > hist1
        cand0 = jnp.logical_xor(gt0[:-1], gt0[1:]) & (hist0[:-1] > 0)
        gt1 = hist1 > hist0
        cand1 = jnp.logical_xor(gt1[:-1], gt1[1:]) & (hist1[:-1] > 0)
        mask = jnp.zeros((E,), bool).at[1:N_BINS].set(cand0 | cand1)
        cnt = jnp.sum(mask)
        mask = mask.at[-1].set(mask[-1] | (cnt == 1))

        a_c = -jnp.log1p(jnp.float32(-EPS))
        b_c = -jnp.log(jnp.float32(EPS))

        def bce(correct):
            return ((n_f - correct) * b_c + correct * a_c) / n_f

        c0 = ns_le_j + (Nb - nb_le_j)
        c1 = (Ns - ns_lt_j) + nb_lt_j
        c2 = (ns_le_j[None, :] - ns_lt_j[:, None]) + Nb - (
            nb_le_j[None, :] - nb_lt_j[:, None]
        )
        c3 = ns_le_j[:, None] + (Ns - ns_lt_j[None, :]) + (
            nb_le_j[None, :] - nb_lt_j[:, None]
        )

        L = jnp.stack(
            [
                jnp.broadcast_to(bce(c0)[:, None], (E, E)),
                jnp.broadcast_to(bce(c1)[:, None], (E, E)),
                bce(c2),
                bce(c3),
            ]
        )
        per_pair_min = jnp.min(L, axis=0)
        per_pair_case = jnp.argmin(L, axis=0)

        idxs = jnp.arange(E)
        valid = mask[:, None] & mask[None, :] & (idxs[:, None] < idxs[None, :])
        flat = jnp.argmin(jnp.where(valid, per_pair_min, jnp.inf))
        i = int(flat) // E
        j = int(flat) % E
        lower = np.float32(edges[i])
        upper = np.float32(edges[j])
        case = int(per_pair_case[i, j])

    # ---- L2: predicate on bf16 grid (original order) ----------------------
    lo_cmp = np.float32(BF(lower))
    up_cmp = np.float32(BF(upper))
    prm = np.zeros((P, 8), np.float32)
    prm[:, 0] = lo_cmp
    prm[:, 1] = up_cmp

    res3 = _run(
        f"pred{case}",
        [
            {"x": xb_all[c * CORE_N : c * CORE_N + DEV_N], "prm": prm}
            for c in CORE_IDS
        ],
    )

    out = np.empty(N, np.int32)
    for c in CORE_IDS:
        out[c * CORE_N : c * CORE_N + DEV_N] = (
            res3[c]["pred"].reshape(-1).astype(np.float32) != 0.0
        )
        # layout tail: exact on host
        tx = x[c * CORE_N + DEV_N : (c + 1) * CORE_N]
        if case == 0:
            tp = tx <= lower
        elif case == 1:
            tp = tx >= lower
        elif case == 2:
            tp = (tx >= lower) & (tx <= upper)
        else:
            tp = (tx <= lower) | (tx >= upper)
        out[c * CORE_N + DEV_N : (c + 1) * CORE_N] = tp.astype(np.int32)

    # repair events within bf16 rounding of the cuts (exact fp32 predicate)
    tol_lo = 6.0 / 128.0 * (abs(float(lower)) + 0.01)
    tol_up = 6.0 / 128.0 * (abs(float(upper)) + 0.01)
    rep = (np.abs(x - lower) <= tol_lo) | (np.abs(x - upper) <= tol_up)
    ri = np.flatnonzero(rep)
    rx = x[ri]
    if case == 0:
        rp = rx <= lower
    elif case == 1:
        rp = rx >= lower
    elif case == 2:
        rp = (rx >= lower) & (rx <= upper)
    else:
        rp = (rx <= lower) | (rx >= upper)
    out[ri] = rp.astype(np.int32)
    return out


# revision 12
# speedup vs baseline: 1.4157x; 1.0825x over previous
"""Trainium2 Bass kernel for nn_CutLayer (histogram_binning).

Two device launches over 8 cores (data-parallel on events):

L1 "counts": per-core class-compacted bf16 tile [128, F2] (rows 0-63 =
  signal events, rows 64-127 = background, pads = +BIG). 51 bf16-grid
  edge thresholds are counted by three engine paths running concurrently:
    - PE path (N_B edges): DVE plain tensor_scalar compare at 4x rate
      (bf16) into ping-pong buffers; PE reduces each compare tile with a
      one-hot stationary matmul into PSUM rows [2 per edge], accumulated
      across 512-column chunks; one final DVE reduce -> [2*N_B, 1].
    - DVE-accum path (N_A edges): fused tensor_scalar compare+accum (1x).
    - ACT path (N_C edges): Sign activation with bias strictly between
      bf16 grid points (no sign(0) ties) + accumulator.
  Host converts to exact fp32-semantics counts (le and lt) by correcting
  a small candidate set of events within a few bf16 ulps of each edge,
  then replicates the reference's pair search bit-exactly (eager CPU jax)
  to produce (lower, upper, case).

L2 "pred": case-specialized bf16 compare(s) on the original-order
  events; host flips the few events within bf16 rounding of the chosen
  cuts and handles the 512-event layout tail exactly.

Host work is O(N) numpy prep/fixup only: min/max, bf16 casts, class
compaction, candidate repair, tiny 51x51 pair search.
"""

from contextlib import ExitStack

import numpy as np
import ml_dtypes

import concourse.bass as bass
import concourse.mybir as mybir
from concourse.bass_utils import run_bass_kernel_spmd

N = 8_000_000
N_CORES = 8
CORE_N = N // N_CORES            # 1_000_000
P = 128
HP = 64                          # rows per class in the counts tile
N_BINS = 50
E = N_BINS + 1                   # 51 edges
EPS = 1e-7
BIG = np.float32(1.0e30)         # bf16-exact sentinel above every edge

# pred layout (original order)
FP = 7812
DEV_N = P * FP                   # 999_936 device events/core for pred

# counts engine split (sums to E)
N_B = 28                         # DVE compare -> PE matmul reduce
N_A = 7                          # DVE fused compare+accum
N_C = 16                         # ACT sign path
N_F = 0                          # of the N_B edges, pool-folded ones
NCB = 4                          # compare-buffer ping-pong depth
CH = 512                         # psum chunk columns

FP32 = mybir.dt.float32
BF16 = mybir.dt.bfloat16
I32 = mybir.dt.int32
AX = mybir.AxisListType
OP = mybir.AluOpType
ACT = mybir.ActivationFunctionType

CORE_IDS = list(range(N_CORES))
BF = ml_dtypes.bfloat16


# --------------------------------------------------------------------------
# Bass programs
# --------------------------------------------------------------------------

def _build_counts_v2(F2: int):
    nchunks = (F2 + CH - 1) // CH
    H2 = F2 // 2
    nhalf = (H2 + CH - 1) // CH
    MROWS = 2 * N_B
    # which B-edges are pool-folded (PE does half the matmuls on those)
    fold = [(i * N_F) // N_B > ((i - 1) * N_F) // N_B for i in range(N_B)]
    fidx = {}
    uidx = {}
    for i in range(N_B):
        if fold[i]:
            fidx[i] = len(fidx)
        else:
            uidx[i] = len(uidx)
    nc = bass.Bass()
    xt_in = nc.declare_dram_parameter("xt", [P, F2], BF16, isOutput=False)
    ed = nc.declare_dram_parameter("edges", [P, 2 * E], FP32, isOutput=False)
    oh_in = nc.declare_dram_parameter("oh", [P, N_B * MROWS], BF16,
                                      isOutput=False)
    acca_o = nc.declare_dram_parameter("acca", [P, N_A], FP32, isOutput=True)
    accc_o = nc.declare_dram_parameter("accc", [P, N_C], FP32, isOutput=True)
    accb_o = nc.declare_dram_parameter("accb", [MROWS, 1], FP32, isOutput=True)
    with ExitStack() as es:
        ec = es.enter_context
        xt = ec(nc.sbuf_tensor([P, F2], BF16))
        cb = [ec(nc.sbuf_tensor(f"cb{i}", [P, F2], BF16)) for i in range(NCB)]
        fb = [ec(nc.sbuf_tensor(f"fb{i}", [P, H2], BF16)) for i in range(2)]
        scra = ec(nc.sbuf_tensor([P, F2], BF16))
        scrc = ec(nc.sbuf_tensor([P, F2], BF16))
        edt = ec(nc.sbuf_tensor([P, 2 * E], FP32))
        oht = ec(nc.sbuf_tensor([P, N_B * MROWS], BF16))
        acca = ec(nc.sbuf_tensor([P, N_A], FP32))
        accc = ec(nc.sbuf_tensor([P, N_C], FP32))
        accb = ec(nc.sbuf_tensor([MROWS, 1], FP32))
        ps = ec(nc.psum_tensor([MROWS, CH], FP32))
        dse = ec(nc.semaphore("dse"))
        dsx0 = ec(nc.semaphore("dsx0"))
        dsx1 = ec(nc.semaphore("dsx1"))
        dsx2 = ec(nc.semaphore("dsx2"))
        vprod = ec(nc.semaphore("vprod"))
        ucons = ec(nc.semaphore("ucons"))   # PE consumed unfolded edge
        pfold = ec(nc.semaphore("pfold"))   # pool folded edge f (also frees cb)
        fdone = ec(nc.semaphore("fdone"))   # PE consumed fold f
        adone = ec(nc.semaphore("adone"))
        cdone = ec(nc.semaphore("cdone"))
        bdone = ec(nc.semaphore("bdone"))
        dso = ec(nc.semaphore("dso"))
        block = ec(nc.Block())

        S1 = (int(0.50 * F2) // 2) * 2
        S2 = (int(0.75 * F2) // 2) * 2

        @block.sync
        def _(sync):
            sync.dma_start(oht[:], oh_in[:]).then_inc(dse, 16)
            sync.dma_start(xt[:, S1:S2], xt_in[:, S1:S2]).then_inc(dsx1, 16)
            sync.wait_ge(cdone, 1)
            sync.dma_start(accc_o[:], accc[:]).then_inc(dso, 16)
            sync.wait_ge(adone, 1)
            sync.dma_start(acca_o[:], acca[:]).then_inc(dso, 16)
            sync.wait_ge(bdone, 1)
            sync.dma_start(accb_o[:], accb[:]).then_inc(dso, 16)
            sync.wait_ge(dso, 48)

        @block.scalar
        def _(scalar):
            scalar.dma_start(edt[:], ed[:]).then_inc(dse, 16)
            scalar.dma_start(xt[:, 0:S1], xt_in[:, 0:S1]).then_inc(dsx0, 16)
            scalar.wait_ge(dse, 32)
            scalar.wait_ge(dsx0, 16)
            scalar.wait_ge(dsx1, 16)
            scalar.wait_ge(dsx2, 16)
            for j in range(N_C):
                ins = scalar.activation(
                    scrc[:], xt[:], ACT.Sign, bias=edt[:, E + j : E + j + 1],
                    scale=1.0, accum_out=accc[:, j : j + 1],
                )
                if j == N_C - 1:
                    ins.then_inc(cdone, 1)

        @block.vector
        def _(vector):
            vector.wait_ge(dse, 32)
            vector.wait_ge(dsx0, 16)
            vector.wait_ge(dsx1, 16)
            vector.wait_ge(dsx2, 16)
            na = 0
            last_a_ins = None
            for i in range(N_B):
                j = i - NCB
                if j >= 0:
                    if fold[j]:
                        vector.wait_ge(pfold, fidx[j] + 1)
                    else:
                        vector.wait_ge(ucons, uidx[j] + 1)
                vector.tensor_scalar(
                    cb[i % NCB][:], xt[:], edt[:, N_A + i : N_A + i + 1], None,
                    OP.is_le,
                ).then_inc(vprod, 1)
                if (i % 4 == 3) and na < N_A:
                    last_a_ins = vector.tensor_scalar(
                        scra[:], xt[:], edt[:, na : na + 1], 0.0,
                        OP.is_le, OP.add, accum_out=acca[:, na : na + 1],
                    )
                    na += 1
            while na < N_A:
                last_a_ins = vector.tensor_scalar(
                    scra[:], xt[:], edt[:, na : na + 1], 0.0,
                    OP.is_le, OP.add, accum_out=acca[:, na : na + 1],
                )
                na += 1
            if last_a_ins is not None:
                last_a_ins.then_inc(adone, 1)
            vector.wait_ge(ucons, len(uidx))
            vector.wait_ge(fdone, len(fidx))
            vector.tensor_reduce(
                accb[:, 0:1], ps[:, 0:CH], axis=AX.X, op=OP.add
            ).then_inc(bdone, 1)

        @block.gpsimd
        def _(gpsimd):
            gpsimd.dma_start(xt[:, S2:F2], xt_in[:, S2:F2]).then_inc(dsx2, 16)
            for i in range(N_B):
                if not fold[i]:
                    continue
                f = fidx[i]
                gpsimd.wait_ge(vprod, i + 1)
                if f >= 2:
                    gpsimd.wait_ge(fdone, f - 1)
                gpsimd.tensor_tensor(
                    fb[f % 2][:], cb[i % NCB][:, 0:H2], cb[i % NCB][:, H2:F2],
                    OP.add,
                ).then_inc(pfold, 1)

        @block.tensor
        def _(tensor):
            first = True
            for i in range(N_B):
                w = oht[:, i * MROWS : (i + 1) * MROWS]
                if fold[i]:
                    f = fidx[i]
                    tensor.wait_ge(pfold, f + 1)
                    src_t = fb[f % 2]
                    nck = nhalf
                    cap = H2
                else:
                    tensor.wait_ge(vprod, i + 1)
                    src_t = cb[i % NCB]
                    nck = nchunks
                    cap = F2
                for c in range(nck):
                    c0 = c * CH
                    c1 = min(cap, c0 + CH)
                    ins = tensor.matmul(
                        ps[:, 0 : c1 - c0], w, src_t[:, c0:c1],
                        start=first,
                        stop=(i == N_B - 1 and c == nck - 1),
                        skip_group_check=True,
                    )
                    if c > 0:
                        ins.ins.ldweights = False
                    first = False
                if fold[i]:
                    ins.then_inc(fdone, 1)
                else:
                    ins.then_inc(ucons, 1)
    return nc


def _build_pred(case: int):
    """Case-specialized predicate on bf16 events (original order):
    0: x <= lo ; 1: x >= lo ; 2: (x >= lo) & (x <= up) ;
    3: (x <= lo) | (x >= up)  (disjoint -> add)
    """
    nc = bass.Bass()
    x = nc.declare_dram_parameter("x", [DEV_N], BF16, isOutput=False)
    pr = nc.declare_dram_parameter("prm", [P, 8], FP32, isOutput=False)
    out = nc.declare_dram_parameter("pred", [DEV_N], BF16, isOutput=True)
    HF = FP // 2
    with (
        nc.sbuf_tensor([P, FP], BF16) as xt,
        nc.sbuf_tensor([P, FP], BF16) as t,
        nc.sbuf_tensor([P, FP], BF16) as s,
        nc.sbuf_tensor([P, FP], BF16) as pi,
        nc.sbuf_tensor([P, 8], FP32) as prm,
        nc.semaphore("d0") as d0,
        nc.semaphore("d1") as d1,
        nc.semaphore("csem") as csem,
        nc.semaphore("dso") as dso,
        nc.Block() as block,
    ):
        xv = x[:].rearrange("(p f) -> p f", p=P)
        ov = out[:].rearrange("(p f) -> p f", p=P)

        @block.sync
        def _(sync):
            sync.dma_start(prm[:], pr[:]).then_inc(d0, 16)
            sync.dma_start(xt[:, HF:FP], xv[:, HF:FP]).then_inc(d1, 16)
            sync.wait_ge(csem, 1)
            sync.dma_start(ov[:, 0:HF], pi[:, 0:HF]).then_inc(dso, 16)
            sync.wait_ge(dso, 32)

        @block.scalar
        def _(scalar):
            scalar.dma_start(xt[:, 0:HF], xv[:, 0:HF]).then_inc(d0, 16)
            scalar.wait_ge(csem, 2)
            scalar.dma_start(ov[:, HF:FP], pi[:, HF:FP]).then_inc(dso, 16)

        @block.vector
        def _(vector):
            lo = prm[:, 0:1]
            up = prm[:, 1:2]
            vector.wait_ge(d0, 32)
            for h in (0, 1):
                if h == 1:
                    vector.wait_ge(d1, 16)
                sl = slice(0, HF) if h == 0 else slice(HF, FP)
                if case == 0:
                    vector.tensor_scalar(
                        pi[:, sl], xt[:, sl], lo, None, OP.is_le
                    ).then_inc(csem, 1)
                elif case == 1:
                    vector.tensor_scalar(
                        pi[:, sl], xt[:, sl], lo, None, OP.is_ge
                    ).then_inc(csem, 1)
                elif case == 2:
                    vector.tensor_scalar(t[:, sl], xt[:, sl], up, None,
                                         OP.is_le)
                    vector.tensor_scalar(s[:, sl], xt[:, sl], lo, None,
                                         OP.is_ge)
                    vector.tensor_tensor(
                        pi[:, sl], s[:, sl], t[:, sl], OP.mult
                    ).then_inc(csem, 1)
                else:
                    vector.tensor_scalar(t[:, sl], xt[:, sl], up, None,
                                         OP.is_ge)
                    vector.tensor_scalar(s[:, sl], xt[:, sl], lo, None,
                                         OP.is_le)
                    vector.tensor_tensor(
                        pi[:, sl], s[:, sl], t[:, sl], OP.add
                    ).then_inc(csem, 1)
    return nc


_PROGRAMS: dict = {}


def _prog(name, *args):
    key = (name, args)
    if key not in _PROGRAMS:
        if name == "counts":
            _PROGRAMS[key] = _build_counts_v2(*args)
        else:
            _PROGRAMS[key] = _build_pred(int(name[4:]))
    return _PROGRAMS[key]


LAST_EXEC_NS: list = []
_CACHE_SET = False


def _enable_jit_cache():
    global _CACHE_SET
    if _CACHE_SET:
        return
    _CACHE_SET = True
    try:
        import jax

        jax.config.update("jax_compilation_cache_dir", "/tmp/jax_bass_cache")
        jax.config.update("jax_persistent_cache_min_compile_time_secs", 1.0)
        jax.config.update("jax_persistent_cache_min_entry_size_bytes", 0)
    except Exception:
        pass


def _run(name, in_maps, *args):
    import os

    _enable_jit_cache()
    trace = bool(int(os.environ.get("BASS_KERNEL_PROFILE", "0")))
    r = run_bass_kernel_spmd(_prog(name, *args), in_maps, CORE_IDS, trace=trace)
    if trace:
        LAST_EXEC_NS.append((name, r.exec_time_ns, r.mean_exec_time_ns))
    return r.results


# --------------------------------------------------------------------------
# Host orchestration
# --------------------------------------------------------------------------

def _ulp_quarter(e64):
    """0.25 * (lower bound of the bf16 ulp at e), elementwise, float64."""
    a = np.abs(e64)
    a = np.where(a < 1e-30, 1e-30, a)
    return 0.25 * np.exp2(np.floor(np.log2(a)) - 8.0)


def kernel(inputs: np.ndarray, targets: np.ndarray) -> np.ndarray:
    import jax
    import jax.numpy as jnp

    x = np.ascontiguousarray(inputs[:, 0]).astype(np.float32, copy=False)
    y = np.asarray(targets)
    sig = y == 1

    LAST_EXEC_NS.clear()

    # ---- edges (host, bit-exact with the reference) -----------------------
    gmin = np.float32(x.min())
    gmax = np.float32(x.max())
    cpu = jax.devices("cpu")[0]
    with jax.default_device(cpu):
        edges = np.asarray(jnp.linspace(jnp.float32(gmin), jnp.float32(gmax), E))
    e_cmp = edges.astype(BF).astype(np.float32)          # bf16-grid thresholds
    e_act64 = e_cmp.astype(np.float64) + _ulp_quarter(e_cmp.astype(np.float64))
    e_act = e_act64.astype(np.float32)                   # strictly between grid pts

    # ---- class-compacted bf16 tiles --------------------------------------
    xb_all = x.astype(BF)
    xs_sig = xb_all[sig]
    xs_bg = xb_all[~sig]
    Ns_i, Nb_i = xs_sig.size, xs_bg.size
    cap_rows = N_CORES * HP
    F2 = (max(Ns_i, Nb_i) + cap_rows - 1) // cap_rows
    if F2 % 2:
        F2 += 1
    big_bf = BF(BIG)
    sig_pad = np.full(cap_rows * F2, big_bf, dtype=BF)
    sig_pad[:Ns_i] = xs_sig
    bg_pad = np.full(cap_rows * F2, big_bf, dtype=BF)
    bg_pad[:Nb_i] = xs_bg
    sig_tiles = sig_pad.reshape(N_CORES, HP, F2)
    bg_tiles = bg_pad.reshape(N_CORES, HP, F2)

    # edge -> slot assignment: A slots 0..N_A-1, B slots N_A..N_A+N_B-1 use
    # e_cmp; C slots use -e_act. Keep natural order (all e_act distinct from
    # e_cmp by construction; _ulp_quarter guards tiny magnitudes).
    ed_in = np.zeros((P, 2 * E), np.float32)
    ed_in[:, :E] = np.concatenate([e_cmp[:N_A], e_cmp[N_A : N_A + N_B],
                                   np.zeros(E - N_A - N_B, np.float32)])
    ed_in[:, E : E + N_C] = -e_act[N_A + N_B :]

    MROWS = 2 * N_B
    oh = np.zeros((P, N_B * MROWS), BF)
    for i in range(N_B):
        oh[0:HP, i * MROWS + 2 * i] = 1        # signal rows -> psum row 2i
        oh[HP:P, i * MROWS + 2 * i + 1] = 1    # background rows -> 2i+1

    res = _run(
        "counts",
        [
            {
                "xt": np.ascontiguousarray(
                    np.concatenate([sig_tiles[c], bg_tiles[c]], axis=0)
                ),
                "edges": ed_in,
                "oh": oh,
            }
            for c in CORE_IDS
        ],
        F2,
    )

    # ---- decode device counts (counts of xb <= e_cmp[k], per class) ------
    d_sig = np.zeros(E, np.float64)
    d_bg = np.zeros(E, np.float64)
    TOT_HALF = N_CORES * HP * F2
    for r in res:
        a = r["acca"].astype(np.float64)      # [P, N_A]
        c = r["accc"].astype(np.float64)      # [P, N_C]
        b = r["accb"].astype(np.float64)      # [MROWS, 1]
        d_sig[0:N_A] += a[0:HP].sum(axis=0)
        d_bg[0:N_A] += a[HP:P].sum(axis=0)
        d_sig[N_A : N_A + N_B] += b[0::2, 0]
        d_bg[N_A : N_A + N_B] += b[1::2, 0]
        # ACT: S = sum sign(x - e_act); le = (TOT - S)/2 per class half
        d_sig[N_A + N_B :] -= c[0:HP].sum(axis=0) / 2.0
        d_bg[N_A + N_B :] -= c[HP:P].sum(axis=0) / 2.0
    d_sig[N_A + N_B :] += TOT_HALF / 2.0
    d_bg[N_A + N_B :] += TOT_HALF / 2.0
    # pads count as "greater" in every path: subtract nothing for le counts.

    # ---- host repair: exact le/lt counts under fp32 semantics ------------
    h64 = (np.float64(gmax) - np.float64(gmin)) / N_BINS
    inv_h = np.float32(1.0 / h64) if h64 != 0 else np.float32(0.0)
    u = (x - gmin) * inv_h
    k0 = np.rint(u).astype(np.int64)
    tol = (6.0 / 128.0) * (np.abs(edges.astype(np.float64)) + 0.01)
    cand_mask = np.zeros(N, bool)
    for dk in (-1, 0, 1):
        kk = np.clip(k0 + dk, 0, E - 1)
        cand_mask |= np.abs(x.astype(np.float64) - edges[kk]) <= tol[kk]
    ci = np.flatnonzero(cand_mask)
    cx = x[ci]
    cxb = xb_all[ci].astype(np.float32)
    csig = sig[ci]
    ck0 = k0[ci]

    corr_le_sig = np.zeros(E, np.float64)
    corr_le_bg = np.zeros(E, np.float64)
    corr_lt_sig = np.zeros(E, np.float64)
    corr_lt_bg = np.zeros(E, np.float64)
    for dk in (-1, 0, 1):
        kr = ck0 + dk
        use = (kr >= 0) & (kr < E)
        kku = kr[use]
        dev = (cxb[use] <= e_cmp[kku]).astype(np.float64)
        t_le = (cx[use] <= edges[kku]).astype(np.float64)
        t_lt = (cx[use] < edges[kku]).astype(np.float64)
        s_u = csig[use]
        np.add.at(corr_le_sig, kku[s_u], (t_le - dev)[s_u])
        np.add.at(corr_le_bg, kku[~s_u], (t_le - dev)[~s_u])
        np.add.at(corr_lt_sig, kku[s_u], (t_lt - dev)[s_u])
        np.add.at(corr_lt_bg, kku[~s_u], (t_lt - dev)[~s_u])

    ns_le = (d_sig + corr_le_sig).astype(np.float32)
    nb_le = (d_bg + corr_le_bg).astype(np.float32)
    ns_lt = (d_sig + corr_lt_sig).astype(np.float32)
    nb_lt = (d_bg + corr_lt_bg).astype(np.float32)

    # ---- replicate the reference's pair search bit-exactly ----------------
    with jax.default_device(cpu):
        ns_le_j = jnp.asarray(ns_le)
        ns_lt_j = jnp.asarray(ns_lt)
        nb_le_j = jnp.asarray(nb_le)
        nb_lt_j = jnp.asarray(nb_lt)
        n_f = jnp.float32(N)
        Ns = ns_le_j[-1]
        Nb = n_f - Ns

        hist0 = nb_le_j[1:] - nb_lt_j[:-1]
        hist1 = ns_le_j[1:] - ns_lt_j[:-1]

        gt0 = hist0 > hist1
        cand0 = jnp.logical_xor(gt0[:-1], gt0[1:]) & (hist0[:-1] > 0)
        gt1 = hist1 > hist0
        cand1 = jnp.logical_xor(gt1[:-1], gt1[1:]) & (hist1[:-1] > 0)
        mask = jnp.zeros((E,), bool).at[1:N_BINS].set(cand0 | cand1)
        cnt = jnp.sum(mask)
        mask = mask.at[-1].set(mask[-1] | (cnt == 1))

        a_c = -jnp.log1p(jnp.float32(-EPS))
        b_c = -jnp.log(jnp.float32(EPS))

        def bce(correct):
            return ((n_f - correct) * b_c + correct * a_c) / n_f

        c0 = ns_le_j + (Nb - nb_le_j)
        c1 = (Ns - ns_lt_j) + nb_lt_j
        c2 = (ns_le_j[None, :] - ns_lt_j[:, None]) + Nb - (
            nb_le_j[None, :] - nb_lt_j[:, None]
        )
        c3 = ns_le_j[:, None] + (Ns - ns_lt_j[None, :]) + (
            nb_le_j[None, :] - nb_lt_j[:, None]
        )

        L = jnp.stack(
            [
                jnp.broadcast_to(bce(c0)[:, None], (E, E)),
                jnp.broadcast_to(bce(c1)[:, None], (E, E)),
                bce(c2),
                bce(c3),
            ]
        )
        per_pair_min = jnp.min(L, axis=0)
        per_pair_case = jnp.argmin(L, axis=0)

        idxs = jnp.arange(E)
        valid = mask[:, None] & mask[None, :] & (idxs[:, None] < idxs[None, :])
        flat = jnp.argmin(jnp.where(valid, per_pair_min, jnp.inf))
        i = int(flat) // E
        j = int(flat) % E
        lower = np.float32(edges[i])
        upper = np.float32(edges[j])
        case = int(per_pair_case[i, j])

    # ---- L2: predicate on bf16 grid (original order) ----------------------
    lo_cmp = np.float32(BF(lower))
    up_cmp = np.float32(BF(upper))
    prm = np.zeros((P, 8), np.float32)
    prm[:, 0] = lo_cmp
    prm[:, 1] = up_cmp

    res3 = _run(
        f"pred{case}",
        [
            {"x": xb_all[c * CORE_N : c * CORE_N + DEV_N], "prm": prm}
            for c in CORE_IDS
        ],
    )

    out = np.empty(N, np.int32)
    for c in CORE_IDS:
        out[c * CORE_N : c * CORE_N + DEV_N] = (
            res3[c]["pred"].reshape(-1).astype(np.float32) != 0.0
        )
        # layout tail: exact on host
        tx = x[c * CORE_N + DEV_N : (c + 1) * CORE_N]
        if case == 0:
            tp = tx <= lower
        elif case == 1:
            tp = tx >= lower
        elif case == 2:
            tp = (tx >= lower) & (tx <= upper)
        else:
            tp = (tx <= lower) | (tx >= upper)
        out[c * CORE_N + DEV_N : (c + 1) * CORE_N] = tp.astype(np.int32)

    # repair events within bf16 rounding of the cuts (exact fp32 predicate)
    tol_lo = 6.0 / 128.0 * (abs(float(lower)) + 0.01)
    tol_up = 6.0 / 128.0 * (abs(float(upper)) + 0.01)
    rep = (np.abs(x - lower) <= tol_lo) | (np.abs(x - upper) <= tol_up)
    ri = np.flatnonzero(rep)
    rx = x[ri]
    if case == 0:
        rp = rx <= lower
    elif case == 1:
        rp = rx >= lower
    elif case == 2:
        rp = (rx >= lower) & (rx <= upper)
    else:
        rp = (rx <= lower) | (rx >= upper)
    out[ri] = rp.astype(np.int32)
    return out


# revision 13
# speedup vs baseline: 1.4813x; 1.0463x over previous
"""Trainium2 Bass kernel for nn_CutLayer (histogram_binning).

Two device launches over 8 cores (data-parallel on events):

L1 "counts": per-core class-compacted bf16 tile [128, F2] (rows 0-63 =
  signal events, rows 64-127 = background, pads = +BIG). 51 bf16-grid
  edge thresholds are counted by three engine paths running concurrently:
    - PE path (N_B edges): DVE plain tensor_scalar compare at 4x rate
      (bf16) into ping-pong buffers; PE reduces each compare tile with a
      one-hot stationary matmul into PSUM rows [2 per edge], accumulated
      across 512-column chunks; one final DVE reduce -> [2*N_B, 1].
    - DVE-accum path (N_A edges): fused tensor_scalar compare+accum (1x).
    - ACT path (N_C edges): Sign activation with bias strictly between
      bf16 grid points (no sign(0) ties) + accumulator.
  Host converts to exact fp32-semantics counts (le and lt) by correcting
  a small candidate set of events within a few bf16 ulps of each edge,
  then replicates the reference's pair search bit-exactly (eager CPU jax)
  to produce (lower, upper, case).

L2 "pred": case-specialized bf16 compare(s) on the original-order
  events; host flips the few events within bf16 rounding of the chosen
  cuts and handles the 512-event layout tail exactly.

Host work is O(N) numpy prep/fixup only: min/max, bf16 casts, class
compaction, candidate repair, tiny 51x51 pair search.
"""

from contextlib import ExitStack

import numpy as np
import ml_dtypes

import concourse.bass as bass
import concourse.mybir as mybir
from concourse.bass_utils import run_bass_kernel_spmd

N = 8_000_000
N_CORES = 8
CORE_N = N // N_CORES            # 1_000_000
P = 128
HP = 64                          # rows per class in the counts tile
N_BINS = 50
E = N_BINS + 1                   # 51 edges
EPS = 1e-7
BIG = np.float32(1.0e30)         # bf16-exact sentinel above every edge

# pred layout (original order)
FP = 7812
DEV_N = P * FP                   # 999_936 device events/core for pred

# counts engine split (sums to E)
N_B = 28                         # DVE compare -> PE matmul reduce
N_A = 6                          # DVE fused compare+accum
N_C = 17                         # ACT sign path
N_F = 0                          # of the N_B edges, pool-folded ones
NCB = 4                          # compare-buffer ping-pong depth
CH = 512                         # psum chunk columns

FP32 = mybir.dt.float32
BF16 = mybir.dt.bfloat16
I32 = mybir.dt.int32
AX = mybir.AxisListType
OP = mybir.AluOpType
ACT = mybir.ActivationFunctionType

CORE_IDS = list(range(N_CORES))
BF = ml_dtypes.bfloat16


# --------------------------------------------------------------------------
# Bass programs
# --------------------------------------------------------------------------

def _build_counts_v2(F2: int):
    nchunks = (F2 + CH - 1) // CH
    H2 = F2 // 2
    nhalf = (H2 + CH - 1) // CH
    MROWS = 2 * N_B
    # which B-edges are pool-folded (PE does half the matmuls on those)
    fold = [(i * N_F) // N_B > ((i - 1) * N_F) // N_B for i in range(N_B)]
    fidx = {}
    uidx = {}
    for i in range(N_B):
        if fold[i]:
            fidx[i] = len(fidx)
        else:
            uidx[i] = len(uidx)
    nc = bass.Bass()
    xt_in = nc.declare_dram_parameter("xt", [P, F2], BF16, isOutput=False)
    ed = nc.declare_dram_parameter("edges", [P, 2 * E], FP32, isOutput=False)
    oh_in = nc.declare_dram_parameter("oh", [P, N_B * MROWS], BF16,
                                      isOutput=False)
    acca_o = nc.declare_dram_parameter("acca", [P, N_A], FP32, isOutput=True)
    accc_o = nc.declare_dram_parameter("accc", [P, N_C], FP32, isOutput=True)
    accb_o = nc.declare_dram_parameter("accb", [MROWS, 1], FP32, isOutput=True)
    with ExitStack() as es:
        ec = es.enter_context
        xt = ec(nc.sbuf_tensor([P, F2], BF16))
        cb = [ec(nc.sbuf_tensor(f"cb{i}", [P, F2], BF16)) for i in range(NCB)]
        fb = [ec(nc.sbuf_tensor(f"fb{i}", [P, H2], BF16)) for i in range(2)]
        scra = ec(nc.sbuf_tensor([P, F2], BF16))
        scrc = ec(nc.sbuf_tensor([P, F2], BF16))
        edt = ec(nc.sbuf_tensor([P, 2 * E], FP32))
        oht = ec(nc.sbuf_tensor([P, N_B * MROWS], BF16))
        acca = ec(nc.sbuf_tensor([P, N_A], FP32))
        accc = ec(nc.sbuf_tensor([P, N_C], FP32))
        accb = ec(nc.sbuf_tensor([MROWS, 1], FP32))
        ps = ec(nc.psum_tensor([MROWS, CH], FP32))
        dse = ec(nc.semaphore("dse"))
        dsoh = ec(nc.semaphore("dsoh"))
        dsx0 = ec(nc.semaphore("dsx0"))
        dsx1 = ec(nc.semaphore("dsx1"))
        dsx2 = ec(nc.semaphore("dsx2"))
        vprod = ec(nc.semaphore("vprod"))
        ucons = ec(nc.semaphore("ucons"))   # PE consumed unfolded edge
        pfold = ec(nc.semaphore("pfold"))   # pool folded edge f (also frees cb)
        fdone = ec(nc.semaphore("fdone"))   # PE consumed fold f
        adone = ec(nc.semaphore("adone"))
        cdone = ec(nc.semaphore("cdone"))
        bdone = ec(nc.semaphore("bdone"))
        dso = ec(nc.semaphore("dso"))
        block = ec(nc.Block())

        S1 = (int(0.40 * F2) // 2) * 2
        S2 = (int(0.68 * F2) // 2) * 2

        @block.sync
        def _(sync):
            sync.dma_start(xt[:, S1:S2], xt_in[:, S1:S2]).then_inc(dsx1, 16)
            sync.dma_start(oht[:], oh_in[:]).then_inc(dsoh, 16)
            sync.wait_ge(cdone, 1)
            sync.dma_start(accc_o[:], accc[:]).then_inc(dso, 16)
            sync.wait_ge(adone, 1)
            sync.dma_start(acca_o[:], acca[:]).then_inc(dso, 16)
            sync.wait_ge(bdone, 1)
            sync.dma_start(accb_o[:], accb[:]).then_inc(dso, 16)

        @block.scalar
        def _(scalar):
            scalar.dma_start(edt[:], ed[:]).then_inc(dse, 16)
            scalar.dma_start(xt[:, 0:S1], xt_in[:, 0:S1]).then_inc(dsx0, 16)
            scalar.wait_ge(dse, 16)
            scalar.wait_ge(dsx0, 16)
            scalar.wait_ge(dsx1, 16)
            scalar.wait_ge(dsx2, 16)
            for j in range(N_C):
                ins = scalar.activation(
                    scrc[:], xt[:], ACT.Sign, bias=edt[:, E + j : E + j + 1],
                    scale=1.0, accum_out=accc[:, j : j + 1],
                )
                if j == N_C - 1:
                    ins.then_inc(cdone, 1)

        @block.vector
        def _(vector):
            vector.wait_ge(dse, 16)
            vector.wait_ge(dsx0, 16)
            vector.wait_ge(dsx1, 16)
            vector.wait_ge(dsx2, 16)
            na = 0
            last_a_ins = None
            for i in range(N_B):
                j = i - NCB
                if j >= 0:
                    if fold[j]:
                        vector.wait_ge(pfold, fidx[j] + 1)
                    else:
                        vector.wait_ge(ucons, uidx[j] + 1)
                vector.tensor_scalar(
                    cb[i % NCB][:], xt[:], edt[:, N_A + i : N_A + i + 1], None,
                    OP.is_le,
                ).then_inc(vprod, 1)
                if (i % 4 == 3) and na < N_A:
                    last_a_ins = vector.tensor_scalar(
                        scra[:], xt[:], edt[:, na : na + 1], 0.0,
                        OP.is_le, OP.add, accum_out=acca[:, na : na + 1],
                    )
                    na += 1
            while na < N_A:
                last_a_ins = vector.tensor_scalar(
                    scra[:], xt[:], edt[:, na : na + 1], 0.0,
                    OP.is_le, OP.add, accum_out=acca[:, na : na + 1],
                )
                na += 1
            if last_a_ins is not None:
                last_a_ins.then_inc(adone, 1)
            vector.wait_ge(ucons, len(uidx))
            vector.wait_ge(fdone, len(fidx))
            vector.tensor_reduce(
                accb[:, 0:1], ps[:, 0:CH], axis=AX.X, op=OP.add
            ).then_inc(bdone, 1)

        @block.gpsimd
        def _(gpsimd):
            gpsimd.dma_start(xt[:, S2:F2], xt_in[:, S2:F2]).then_inc(dsx2, 16)
            for i in range(N_B):
                if not fold[i]:
                    continue
                f = fidx[i]
                gpsimd.wait_ge(vprod, i + 1)
                if f >= 2:
                    gpsimd.wait_ge(fdone, f - 1)
                gpsimd.tensor_tensor(
                    fb[f % 2][:], cb[i % NCB][:, 0:H2], cb[i % NCB][:, H2:F2],
                    OP.add,
                ).then_inc(pfold, 1)

        @block.tensor
        def _(tensor):
            tensor.wait_ge(dsoh, 16)
            first = True
            for i in range(N_B):
                w = oht[:, i * MROWS : (i + 1) * MROWS]
                if fold[i]:
                    f = fidx[i]
                    tensor.wait_ge(pfold, f + 1)
                    src_t = fb[f % 2]
                    nck = nhalf
                    cap = H2
                else:
                    tensor.wait_ge(vprod, i + 1)
                    src_t = cb[i % NCB]
                    nck = nchunks
                    cap = F2
                for c in range(nck):
                    c0 = c * CH
                    c1 = min(cap, c0 + CH)
                    ins = tensor.matmul(
                        ps[:, 0 : c1 - c0], w, src_t[:, c0:c1],
                        start=first,
                        stop=(i == N_B - 1 and c == nck - 1),
                        skip_group_check=True,
                    )
                    if c > 0:
                        ins.ins.ldweights = False
                    first = False
                if fold[i]:
                    ins.then_inc(fdone, 1)
                else:
                    ins.then_inc(ucons, 1)
    return nc


def _build_pred(case: int):
    """Case-specialized predicate on bf16 events (original order):
    0: x <= lo ; 1: x >= lo ; 2: (x >= lo) & (x <= up) ;
    3: (x <= lo) | (x >= up)  (disjoint -> add)
    """
    nc = bass.Bass()
    x = nc.declare_dram_parameter("x", [DEV_N], BF16, isOutput=False)
    pr = nc.declare_dram_parameter("prm", [P, 8], FP32, isOutput=False)
    out = nc.declare_dram_parameter("pred", [DEV_N], BF16, isOutput=True)
    HF = FP // 2
    with (
        nc.sbuf_tensor([P, FP], BF16) as xt,
        nc.sbuf_tensor([P, FP], BF16) as t,
        nc.sbuf_tensor([P, FP], BF16) as s,
        nc.sbuf_tensor([P, FP], BF16) as pi,
        nc.sbuf_tensor([P, 8], FP32) as prm,
        nc.semaphore("d0") as d0,
        nc.semaphore("d1") as d1,
        nc.semaphore("csem") as csem,
        nc.semaphore("dso") as dso,
        nc.Block() as block,
    ):
        xv = x[:].rearrange("(p f) -> p f", p=P)
        ov = out[:].rearrange("(p f) -> p f", p=P)

        @block.sync
        def _(sync):
            sync.dma_start(prm[:], pr[:]).then_inc(d0, 16)
            sync.dma_start(xt[:, HF:FP], xv[:, HF:FP]).then_inc(d1, 16)
            sync.wait_ge(csem, 1)
            sync.dma_start(ov[:, 0:HF], pi[:, 0:HF]).then_inc(dso, 16)

        @block.scalar
        def _(scalar):
            scalar.dma_start(xt[:, 0:HF], xv[:, 0:HF]).then_inc(d0, 16)
            scalar.wait_ge(csem, 2)
            scalar.dma_start(ov[:, HF:FP], pi[:, HF:FP]).then_inc(dso, 16)

        @block.vector
        def _(vector):
            lo = prm[:, 0:1]
            up = prm[:, 1:2]
            vector.wait_ge(d0, 32)
            for h in (0, 1):
                if h == 1:
                    vector.wait_ge(d1, 16)
                sl = slice(0, HF) if h == 0 else slice(HF, FP)
                if case == 0:
                    vector.tensor_scalar(
                        pi[:, sl], xt[:, sl], lo, None, OP.is_le
                    ).then_inc(csem, 1)
                elif case == 1:
                    vector.tensor_scalar(
                        pi[:, sl], xt[:, sl], lo, None, OP.is_ge
                    ).then_inc(csem, 1)
                elif case == 2:
                    vector.tensor_scalar(t[:, sl], xt[:, sl], up, None,
                                         OP.is_le)
                    vector.tensor_scalar(s[:, sl], xt[:, sl], lo, None,
                                         OP.is_ge)
                    vector.tensor_tensor(
                        pi[:, sl], s[:, sl], t[:, sl], OP.mult
                    ).then_inc(csem, 1)
                else:
                    vector.tensor_scalar(t[:, sl], xt[:, sl], up, None,
                                         OP.is_ge)
                    vector.tensor_scalar(s[:, sl], xt[:, sl], lo, None,
                                         OP.is_le)
                    vector.tensor_tensor(
                        pi[:, sl], s[:, sl], t[:, sl], OP.add
                    ).then_inc(csem, 1)
    return nc


_PROGRAMS: dict = {}


def _prog(name, *args):
    key = (name, args)
    if key not in _PROGRAMS:
        if name == "counts":
            _PROGRAMS[key] = _build_counts_v2(*args)
        else:
            _PROGRAMS[key] = _build_pred(int(name[4:]))
    return _PROGRAMS[key]


LAST_EXEC_NS: list = []
_CACHE_SET = False


def _enable_jit_cache():
    global _CACHE_SET
    if _CACHE_SET:
        return
    _CACHE_SET = True
    try:
        import jax

        jax.config.update("jax_compilation_cache_dir", "/tmp/jax_bass_cache")
        jax.config.update("jax_persistent_cache_min_compile_time_secs", 1.0)
        jax.config.update("jax_persistent_cache_min_entry_size_bytes", 0)
    except Exception:
        pass


def _run(name, in_maps, *args):
    import os

    _enable_jit_cache()
    trace = bool(int(os.environ.get("BASS_KERNEL_PROFILE", "0")))
    r = run_bass_kernel_spmd(_prog(name, *args), in_maps, CORE_IDS, trace=trace)
    if trace:
        LAST_EXEC_NS.append((name, r.exec_time_ns, r.mean_exec_time_ns))
    return r.results


# --------------------------------------------------------------------------
# Host orchestration
# --------------------------------------------------------------------------

def _ulp_quarter(e64):
    """0.25 * (lower bound of the bf16 ulp at e), elementwise, float64."""
    a = np.abs(e64)
    a = np.where(a < 1e-30, 1e-30, a)
    return 0.25 * np.exp2(np.floor(np.log2(a)) - 8.0)


def kernel(inputs: np.ndarray, targets: np.ndarray) -> np.ndarray:
    import jax
    import jax.numpy as jnp

    x = np.ascontiguousarray(inputs[:, 0]).astype(np.float32, copy=False)
    y = np.asarray(targets)
    sig = y == 1

    LAST_EXEC_NS.clear()

    # ---- edges (host, bit-exact with the reference) -----------------------
    gmin = np.float32(x.min())
    gmax = np.float32(x.max())
    cpu = jax.devices("cpu")[0]
    with jax.default_device(cpu):
        edges = np.asarray(jnp.linspace(jnp.float32(gmin), jnp.float32(gmax), E))
    e_cmp = edges.astype(BF).astype(np.float32)          # bf16-grid thresholds
    e_act64 = e_cmp.astype(np.float64) + _ulp_quarter(e_cmp.astype(np.float64))
    e_act = e_act64.astype(np.float32)                   # strictly between grid pts

    # ---- class-compacted bf16 tiles --------------------------------------
    xb_all = x.astype(BF)
    xs_sig = xb_all[sig]
    xs_bg = xb_all[~sig]
    Ns_i, Nb_i = xs_sig.size, xs_bg.size
    cap_rows = N_CORES * HP
    F2 = (max(Ns_i, Nb_i) + cap_rows - 1) // cap_rows
    if F2 % 2:
        F2 += 1
    big_bf = BF(BIG)
    sig_pad = np.full(cap_rows * F2, big_bf, dtype=BF)
    sig_pad[:Ns_i] = xs_sig
    bg_pad = np.full(cap_rows * F2, big_bf, dtype=BF)
    bg_pad[:Nb_i] = xs_bg
    sig_tiles = sig_pad.reshape(N_CORES, HP, F2)
    bg_tiles = bg_pad.reshape(N_CORES, HP, F2)

    # edge -> slot assignment: A slots 0..N_A-1, B slots N_A..N_A+N_B-1 use
    # e_cmp; C slots use -e_act. Keep natural order (all e_act distinct from
    # e_cmp by construction; _ulp_quarter guards tiny magnitudes).
    ed_in = np.zeros((P, 2 * E), np.float32)
    ed_in[:, :E] = np.concatenate([e_cmp[:N_A], e_cmp[N_A : N_A + N_B],
                                   np.zeros(E - N_A - N_B, np.float32)])
    ed_in[:, E : E + N_C] = -e_act[N_A + N_B :]

    MROWS = 2 * N_B
    oh = np.zeros((P, N_B * MROWS), BF)
    for i in range(N_B):
        oh[0:HP, i * MROWS + 2 * i] = 1        # signal rows -> psum row 2i
        oh[HP:P, i * MROWS + 2 * i + 1] = 1    # background rows -> 2i+1

    res = _run(
        "counts",
        [
            {
                "xt": np.ascontiguousarray(
                    np.concatenate([sig_tiles[c], bg_tiles[c]], axis=0)
                ),
                "edges": ed_in,
                "oh": oh,
            }
            for c in CORE_IDS
        ],
        F2,
    )

    # ---- decode device counts (counts of xb <= e_cmp[k], per class) ------
    d_sig = np.zeros(E, np.float64)
    d_bg = np.zeros(E, np.float64)
    TOT_HALF = N_CORES * HP * F2
    for r in res:
        a = r["acca"].astype(np.float64)      # [P, N_A]
        c = r["accc"].astype(np.float64)      # [P, N_C]
        b = r["accb"].astype(np.float64)      # [MROWS, 1]
        d_sig[0:N_A] += a[0:HP].sum(axis=0)
        d_bg[0:N_A] += a[HP:P].sum(axis=0)
        d_sig[N_A : N_A + N_B] += b[0::2, 0]
        d_bg[N_A : N_A + N_B] += b[1::2, 0]
        # ACT: S = sum sign(x - e_act); le = (TOT - S)/2 per class half
        d_sig[N_A + N_B :] -= c[0:HP].sum(axis=0) / 2.0
        d_bg[N_A + N_B :] -= c[HP:P].sum(axis=0) / 2.0
    d_sig[N_A + N_B :] += TOT_HALF / 2.0
    d_bg[N_A + N_B :] += TOT_HALF / 2.0
    # pads count as "greater" in every path: subtract nothing for le counts.

    # ---- host repair: exact le/lt counts under fp32 semantics ------------
    h64 = (np.float64(gmax) - np.float64(gmin)) / N_BINS
    inv_h = np.float32(1.0 / h64) if h64 != 0 else np.float32(0.0)
    u = (x - gmin) * inv_h
    k0 = np.rint(u).astype(np.int64)
    tol = (6.0 / 128.0) * (np.abs(edges.astype(np.float64)) + 0.01)
    cand_mask = np.zeros(N, bool)
    for dk in (-1, 0, 1):
        kk = np.clip(k0 + dk, 0, E - 1)
        cand_mask |= np.abs(x.astype(np.float64) - edges[kk]) <= tol[kk]
    ci = np.flatnonzero(cand_mask)
    cx = x[ci]
    cxb = xb_all[ci].astype(np.float32)
    csig = sig[ci]
    ck0 = k0[ci]

    corr_le_sig = np.zeros(E, np.float64)
    corr_le_bg = np.zeros(E, np.float64)
    corr_lt_sig = np.zeros(E, np.float64)
    corr_lt_bg = np.zeros(E, np.float64)
    for dk in (-1, 0, 1):
        kr = ck0 + dk
        use = (kr >= 0) & (kr < E)
        kku = kr[use]
        dev = (cxb[use] <= e_cmp[kku]).astype(np.float64)
        t_le = (cx[use] <= edges[kku]).astype(np.float64)
        t_lt = (cx[use] < edges[kku]).astype(np.float64)
        s_u = csig[use]
        np.add.at(corr_le_sig, kku[s_u], (t_le - dev)[s_u])
        np.add.at(corr_le_bg, kku[~s_u], (t_le - dev)[~s_u])
        np.add.at(corr_lt_sig, kku[s_u], (t_lt - dev)[s_u])
        np.add.at(corr_lt_bg, kku[~s_u], (t_lt - dev)[~s_u])

    ns_le = (d_sig + corr_le_sig).astype(np.float32)
    nb_le = (d_bg + corr_le_bg).astype(np.float32)
    ns_lt = (d_sig + corr_lt_sig).astype(np.float32)
    nb_lt = (d_bg + corr_lt_bg).astype(np.float32)

    # ---- replicate the reference's pair search bit-exactly ----------------
    with jax.default_device(cpu):
        ns_le_j = jnp.asarray(ns_le)
        ns_lt_j = jnp.asarray(ns_lt)
        nb_le_j = jnp.asarray(nb_le)
        nb_lt_j = jnp.asarray(nb_lt)
        n_f = jnp.float32(N)
        Ns = ns_le_j[-1]
        Nb = n_f - Ns

        hist0 = nb_le_j[1:] - nb_lt_j[:-1]
        hist1 = ns_le_j[1:] - ns_lt_j[:-1]

        gt0 = hist0 > hist1
        cand0 = jnp.logical_xor(gt0[:-1], gt0[1:]) & (hist0[:-1] > 0)
        gt1 = hist1 > hist0
        cand1 = jnp.logical_xor(gt1[:-1], gt1[1:]) & (hist1[:-1] > 0)
        mask = jnp.zeros((E,), bool).at[1:N_BINS].set(cand0 | cand1)
        cnt = jnp.sum(mask)
        mask = mask.at[-1].set(mask[-1] | (cnt == 1))

        a_c = -jnp.log1p(jnp.float32(-EPS))
        b_c = -jnp.log(jnp.float32(EPS))

        def bce(correct):
            return ((n_f - correct) * b_c + correct * a_c) / n_f

        c0 = ns_le_j + (Nb - nb_le_j)
        c1 = (Ns - ns_lt_j) + nb_lt_j
        c2 = (ns_le_j[None, :] - ns_lt_j[:, None]) + Nb - (
            nb_le_j[None, :] - nb_lt_j[:, None]
        )
        c3 = ns_le_j[:, None] + (Ns - ns_lt_j[None, :]) + (
            nb_le_j[None, :] - nb_lt_j[:, None]
        )

        L = jnp.stack(
            [
                jnp.broadcast_to(bce(c0)[:, None], (E, E)),
                jnp.broadcast_to(bce(c1)[:, None], (E, E)),
                bce(c2),
                bce(c3),
            ]
        )
        per_pair_min = jnp.min(L, axis=0)
        per_pair_case = jnp.argmin(L, axis=0)

        idxs = jnp.arange(E)
        valid = mask[:, None] & mask[None, :] & (idxs[:, None] < idxs[None, :])
        flat = jnp.argmin(jnp.where(valid, per_pair_min, jnp.inf))
        i = int(flat) // E
        j = int(flat) % E
        lower = np.float32(edges[i])
        upper = np.float32(edges[j])
        case = int(per_pair_case[i, j])

    # ---- L2: predicate on bf16 grid (original order) ----------------------
    lo_cmp = np.float32(BF(lower))
    up_cmp = np.float32(BF(upper))
    prm = np.zeros((P, 8), np.float32)
    prm[:, 0] = lo_cmp
    prm[:, 1] = up_cmp

    res3 = _run(
        f"pred{case}",
        [
            {"x": xb_all[c * CORE_N : c * CORE_N + DEV_N], "prm": prm}
            for c in CORE_IDS
        ],
    )

    out = np.empty(N, np.int32)
    for c in CORE_IDS:
        out[c * CORE_N : c * CORE_N + DEV_N] = (
            res3[c]["pred"].reshape(-1).astype(np.float32) != 0.0
        )
        # layout tail: exact on host
        tx = x[c * CORE_N + DEV_N : (c + 1) * CORE_N]
        if case == 0:
            tp = tx <= lower
        elif case == 1:
            tp = tx >= lower
        elif case == 2:
            tp = (tx >= lower) & (tx <= upper)
        else:
            tp = (tx <= lower) | (tx >= upper)
        out[c * CORE_N + DEV_N : (c + 1) * CORE_N] = tp.astype(np.int32)

    # repair events within bf16 rounding of the cuts (exact fp32 predicate)
    tol_lo = 6.0 / 128.0 * (abs(float(lower)) + 0.01)
    tol_up = 6.0 / 128.0 * (abs(float(upper)) + 0.01)
    rep = (np.abs(x - lower) <= tol_lo) | (np.abs(x - upper) <= tol_up)
    ri = np.flatnonzero(rep)
    rx = x[ri]
    if case == 0:
        rp = rx <= lower
    elif case == 1:
        rp = rx >= lower
    elif case == 2:
        rp = (rx >= lower) & (rx <= upper)
    else:
        rp = (rx <= lower) | (rx >= upper)
    out[ri] = rp.astype(np.int32)
    return out
